# revision 22
# baseline (speedup 1.0000x reference)
"""Trainium2 Bass kernel for nn_PlatonicConv (linear-attention GNN message passing).

Math (reference):
  q = rope(x@Wq + bq, phase);  k = rope(ones, phase);  v = x@Wv + bv
  phase[n, g, p] = pos[n, :] . freqs[g, 0, p, :]
  KV_b[g] = (1/AVG) * sum_{n in graph b} k[n,g,:] (x) v[n,g,:]
  out[n]  = concat_g( q'[n,g,:] @ KV_b[g] ) @ Wo + bo

Device formulation (per core, data-parallel over graphs; 8 graphs/core):
  - host precomputes trig (cos/sin of phase, feature-major) and
    k = rope(ones) (pre-tiled per graph, 1/sqrt2 folded into Wo scale).
  - Per graph b:  M_b = stack_g(KV_b[g] @ Wo[g-rows]) : [384, 384]
    out[n] = q'[n] @ M_{b(n)}  (+ bo on host).
  - q'/M_b rows use "A-order" over rope pairs t = g*16+p:
      rows   0:128 = E_t (even dims), t=0..127
      rows 128:256 = O_t (odd  dims), t=0..127
      rows 256:384 = E_t|O_t, t=128..191 (merged 128-part tile q2)
  - graphs get variable 128-aligned slots sized to their node count, so
    no Q/V/rope/out work is spent on padding beyond round-to-128.
  - schedule: per chunk ch emit [chunk ch Q/V/rope] [KV+arena of graphs
    ending at ch] [Mb+out of graphs ending at ch-1]; a full chunk of
    matmuls separates each graph's KV from its Mb consumers so PE never
    waits on the elementwise engines.
  - rope q2 combines run on GpSimd only, with products from Vector
    (concurrent V+GpSimd tensor ops contend ~4x, so GpSimd gets the
    minimal tail of the chain).

Self-contained: hardcodes shapes; shards/pads on host inside kernel().
"""

import math
import os
from contextlib import ExitStack

import ml_dtypes
import numpy as np

import concourse.bacc as bacc_mod
import concourse.bass as bass
import concourse.mybir as mybir
import concourse.tile as tile
from concourse.bass_utils import run_bass_kernel_spmd


def _ensure_ntff_hook():
    """Register the axon NTFF profile hook if the image's antenv lacks it."""
    try:
        import antenv.axon_hooks  # noqa: F401

        return True
    except ImportError:
        pass
    try:
        import sys
        import types

        import antenv
        from trn_agent_boot.trn_boot import _ntff_profile_via_ctypes

        mod = types.ModuleType("antenv.axon_hooks")
        _hook = [None]
        mod.set_axon_ntff_profile_hook = lambda h: _hook.__setitem__(0, h)
        mod.get_axon_ntff_profile_hook = lambda: _hook[0]
        sys.modules["antenv.axon_hooks"] = mod
        antenv.axon_hooks = mod
        mod.set_axon_ntff_profile_hook(
            _ntff_profile_via_ctypes("/opt/axon/libaxon_pjrt.so")
        )
        return True
    except Exception:
        return False


FP32 = mybir.dt.float32
BF16 = mybir.dt.bfloat16
AF = mybir.ActivationFunctionType

N = 32768
C = 384
E = 384
G = 12
D = 32
P = 16
SD = 3
NUM_GRAPHS = 64
NCORES = 8
GPD = NUM_GRAPHS // NCORES  # graphs per device
AVG = float(N) / NUM_GRAPHS  # 512.0
NT = 192  # rope pairs = G*P
W = 512  # streaming window


def _a_order_cols():
    """perm such that A-order column r is original q-dim perm[r]."""
    perm = np.empty(E, dtype=np.int64)
    for r in range(E):
        if r < 128:
            t, odd = r, 0
        elif r < 256:
            t, odd = r - 128, 1
        elif r < 320:
            t, odd = 128 + (r - 256), 0
        else:
            t, odd = 128 + (r - 320), 1
        perm[r] = (t // 16) * 32 + 2 * (t % 16) + odd
    return perm


_APERM = _a_order_cols()

_CACHE = {}


def _build(slots: tuple, has_bias: bool):
    """slots: per-graph 128-aligned node capacities (0 = skip graph)."""
    key = (slots, has_bias)
    if key in _CACHE:
        return _CACHE[key]

    offs = [0]
    for s in slots:
        offs.append(offs[-1] + s)
    NUSED = offs[-1]
    NP = ((NUSED + W - 1) // W) * W
    NTILE = NP // 128
    NCH = NP // W
    TPSL = [s // 128 for s in slots]
    SLOTMAX = max(slots)
    KEOFF = [0]
    for t in TPSL:
        KEOFF.append(KEOFF[-1] + t * E)

    nc = bacc_mod.Bacc()

    nk = 4 if has_bias else 3

    xa_d = nc.declare_dram_parameter("xa", [NCH, 128, 3 * W], BF16, isOutput=False)
    tr_d = nc.declare_dram_parameter("tr", [NCH, 128, 4 * W], BF16, isOutput=False)
    xb_d = None
    if has_bias:
        xb_d = nc.declare_dram_parameter("xb", [1, NP], BF16, isOutput=False)
    kn_d = nc.declare_dram_parameter("kn", [128, KEOFF[-1]], BF16, isOutput=False)
    wq_d = nc.declare_dram_parameter("wq", [128, 3 * E], BF16, isOutput=False)
    wv_d = nc.declare_dram_parameter("wv", [128, 3 * E], BF16, isOutput=False)
    wo_d = nc.declare_dram_parameter("wo", [128, 3 * C], BF16, isOutput=False)
    if has_bias:
        wqb_d = nc.declare_dram_parameter("wqb", [1, E], BF16, isOutput=False)
        wvb_d = nc.declare_dram_parameter("wvb", [1, E], BF16, isOutput=False)
    out_d = nc.declare_dram_parameter("outt", [128, 3 * NUSED], BF16, isOutput=True)

    with ExitStack() as ctx:
        tc = ctx.enter_context(tile.TileContext(nc))

        consts = ctx.enter_context(tc.tile_pool(name="consts", bufs=1))
        xtp = ctx.enter_context(tc.tile_pool(name="xtp", bufs=3))
        qsb = ctx.enter_context(tc.tile_pool(name="qsb", bufs=2))
        big = ctx.enter_context(tc.tile_pool(name="big", bufs=1))
        aren = ctx.enter_context(tc.tile_pool(name="aren", bufs=1))
        mbp = ctx.enter_context(tc.tile_pool(name="mbp", bufs=2))
        outp = ctx.enter_context(tc.tile_pool(name="outp", bufs=2))
        kp = ctx.enter_context(tc.tile_pool(name="kp", bufs=2))
        psum = ctx.enter_context(tc.tile_pool(name="psum", bufs=1, space="PSUM"))

        # ---- constants (weights); wq/wv issued via ACT's DGE so the sync
        # engine can trigger chunk-0 input DMAs concurrently ----
        wq_sb = consts.tile([128, 3, E], BF16, tag="wq")
        nc.scalar.dma_start(wq_sb[:], wq_d[:].rearrange("p (k e) -> p k e", k=3))
        wv_sb = consts.tile([128, 3, E], BF16, tag="wv")
        nc.scalar.dma_start(wv_sb[:], wv_d[:].rearrange("p (k e) -> p k e", k=3))
        wo_sb = consts.tile([128, 3, C], BF16, tag="wo")
        if has_bias:
            wqb = consts.tile([1, E], BF16, tag="wqb")
            nc.scalar.dma_start(wqb[:], wqb_d[:])
            wvb = consts.tile([1, E], BF16, tag="wvb")
            nc.scalar.dma_start(wvb[:], wvb_d[:])

        def load_wo():
            nc.sync.dma_start(wo_sb[:], wo_d[:].rearrange("p (k e) -> p k e", k=3))

        def wq_blk(ki, c0, m):
            if ki < 3:
                return wq_sb[:, ki, c0 : c0 + m]
            return wqb[:, c0 : c0 + m]

        def wv_blk(ki):
            if ki < 3:
                return wv_sb[:, ki, :]
            return wvb[:]

        def wos_blk(bi):  # [64, C] block bi (0..5); odd blocks at rows 64:128
            r0 = 64 * (bi % 2)
            return wo_sb[r0 : r0 + 64, bi // 2, :]

        # ---- persistent SBUF tensors ----
        q0 = big.tile([128, NP], BF16, tag="q0")
        q1 = big.tile([128, NP], BF16, tag="q1")
        q2 = big.tile([128, NP], BF16, tag="q2")  # rows 0:64 = E2', 64:128 = O2'
        v_sb = big.tile([128, NTILE, E], BF16, tag="v_sb")

        arenas = []
        for s in range(3):
            row_set = []
            for pr in range(6):
                if pr % 2 == 0:
                    a = aren.tile([64, 64], BF16, tag=f"arena{s}_{pr}")
                else:
                    af = aren.tile(
                        [128, 64], BF16, tag=f"arena{s}_{pr}", name=f"arena{s}_{pr}"
                    )
                    a = af[64:128, :]
                nc.vector.memset(a[:], 0.0)
                row_set.append(a)
            arenas.append(row_set)

        # PE warmup: ~3.5us of dummy matmuls so HAM reaches K=8/8 before
        # the first real chunk (its inputs are still in flight on DMA)
        warm = consts.tile([128, 64], BF16, tag="warm")
        nc.vector.memset(warm[:], 0.0)
        wps = psum.tile([64, 64], FP32, tag="To1", name="To1")
        for _ in range(50):
            nc.tensor.matmul(wps[:], warm[:, 0:64], warm[:], start=True, stop=True)

        # k tile prefetch management
        k_tiles = {}

        def load_k(j):
            tps = TPSL[j]
            if tps == 0:
                return
            kt = kp.tile([128, tps, E], BF16, tag="kt")
            nc.sync.dma_start(
                kt[:],
                kn_d[:, KEOFF[j] : KEOFF[j + 1]].rearrange("p (t e) -> p t e", t=tps),
            )
            k_tiles[j] = kt

        # ------------------------------------------------------------------
        # chunk: Q/V projections + rope for nodes [ch*W, ch*W+W)
        # ------------------------------------------------------------------
        def emit_chunk(ch):
            n0 = ch * W
            xa = xtp.tile([128, 3, W], BF16, tag="xa")
            nc.sync.dma_start(xa[:], xa_d[ch, :, :].rearrange("p (k w) -> p k w", k=3))
            tr = xtp.tile([128, 4, W], BF16, tag="tr")
            nc.sync.dma_start(tr[:], tr_d[ch, :, :].rearrange("p (k w) -> p k w", k=4))
            if has_bias:
                xbt = xtp.tile([1, W], BF16, tag="xbt")
                nc.sync.dma_start(xbt[:], xb_d[:, n0 : n0 + W])

            def x_blk(ki):
                if ki < 3:
                    return xa[:, ki, :]
                return xbt[:]

            clf = tr[:, 0, :]
            slf = tr[:, 1, :]
            clh = tr[0:64, 2, :]
            slh = tr[0:64, 3, :]

            # Q projection (A-ordered columns), 3 psum groups of 128
            qps = []
            for g in range(3):
                ps = psum.tile([128, W], FP32, tag=f"Tq{g}", name=f"Tq{g}")
                c0 = 128 * g
                for ki in range(nk):
                    nc.tensor.matmul(
                        ps[:],
                        wq_blk(ki, c0, 128),
                        x_blk(ki),
                        start=(ki == 0),
                        stop=(ki == nk - 1),
                    )
                qps.append(ps)

            # psum -> SBUF casts
            qE0s = qsb.tile([128, W], BF16, tag="qE0s")
            qO0s = qsb.tile([128, W], BF16, tag="qO0s")
            qE2s = qsb.tile([64, W], BF16, tag="qE2s")
            qO2s = qsb.tile([64, W], BF16, tag="qO2s")
            nc.vector.tensor_copy(qE0s[:], qps[0][:])
            nc.vector.tensor_copy(qO0s[:], qps[1][:])
            nc.scalar.activation(qE2s[:], qps[2][0:64, :], AF.Copy)
            nc.scalar.activation(qO2s[:], qps[2][64:128, :], AF.Copy)

            # rope: independent products then combines
            m1 = qsb.tile([128, W], BF16, tag="m1")
            m2 = qsb.tile([128, W], BF16, tag="m2")
            m3 = qsb.tile([128, W], BF16, tag="m3")
            m4 = qsb.tile([128, W], BF16, tag="m4")
            nc.vector.tensor_mul(m1[:], qE0s[:], clf)
            nc.vector.tensor_mul(m2[:], qO0s[:], slf)
            nc.vector.tensor_mul(m3[:], qE0s[:], slf)
            nc.vector.tensor_mul(m4[:], qO0s[:], clf)
            nc.vector.tensor_sub(q0[:, n0 : n0 + W], m1[:], m2[:])
            nc.vector.tensor_add(q1[:, n0 : n0 + W], m3[:], m4[:])

            eng = nc.gpsimd if ch < NCH - 3 else nc.vector
            n1 = qsb.tile([64, W], BF16, tag="n1")
            n2 = qsb.tile([64, W], BF16, tag="n2")
            n3 = qsb.tile([64, W], BF16, tag="n3")
            n4 = qsb.tile([64, W], BF16, tag="n4")
            nc.vector.tensor_mul(n1[:], qE2s[:], clh)
            nc.vector.tensor_mul(n2[:], qO2s[:], slh)
            nc.vector.tensor_mul(n3[:], qE2s[:], slh)
            nc.vector.tensor_mul(n4[:], qO2s[:], clh)
            eng.tensor_sub(q2[0:64, n0 : n0 + W], n1[:], n2[:])
            eng.tensor_add(q2[64:128, n0 : n0 + W], n3[:], n4[:])

            # V per node tile
            for sub in range(W // 128):
                ti = ch * (W // 128) + sub
                f0 = sub * 128
                vps = psum.tile([128, E], FP32, tag="Tv", name="Tv")
                for ki in range(nk):
                    nc.tensor.matmul(
                        vps[:],
                        x_blk(ki)[:, f0 : f0 + 128],
                        wv_blk(ki),
                        start=(ki == 0),
                        stop=(ki == nk - 1),
                    )
                if sub % 2 == 0:
                    nc.vector.tensor_copy(v_sb[:, ti, :], vps[:])
                else:
                    nc.scalar.activation(v_sb[:, ti, :], vps[:], AF.Copy)

        # ------------------------------------------------------------------
        # graph phase 1: KV + arena copies
        # ------------------------------------------------------------------
        def emit_kv(j, nxt):
            if nxt is not None:
                load_k(nxt)
            tps = TPSL[j]
            kt = k_tiles.pop(j)
            t0 = offs[j] // 128

            kvt = psum.tile([128, 3 * 128], FP32, tag=f"Tkv{j % 2}", name=f"Tkv{j % 2}")
            for cchunk in range(3):
                cs = slice(128 * cchunk, 128 * (cchunk + 1))
                for tt in range(tps):
                    nc.tensor.matmul(
                        kvt[:, cs],
                        v_sb[:, t0 + tt, cs],
                        kt[:, tt, cs],
                        start=(tt == 0),
                        stop=(tt == tps - 1),
                    )

            ars = arenas[j % 3]
            for g in range(G):
                cchunk, m = divmod(g, 4)
                pr, par = divmod(g, 2)
                src = kvt[
                    32 * m : 32 * m + 32,
                    128 * cchunk + 32 * m : 128 * cchunk + 32 * m + 32,
                ]
                dst = ars[pr][32 * par : 32 * par + 32, :].rearrange(
                    "e (h s) -> e h s", s=16
                )[:, par::2, :]
                srcr = src.rearrange("e (h s) -> e h s", s=16)
                if g % 2 == 0:
                    nc.vector.tensor_copy(dst, srcr)
                else:
                    nc.scalar.activation(dst, srcr, AF.Copy)

        # ------------------------------------------------------------------
        # graph phase 2: M_b + out matmuls + output DMA
        # ------------------------------------------------------------------
        def emit_mbout(j, is_last):
            ars = arenas[j % 3]
            mb_ps = []
            for cch in range(2):
                psb = psum.tile([128, C], FP32, tag=f"Tq{cch}", name=f"Tq{cch}")
                colsel = slice(0, 32) if cch == 0 else slice(32, 64)
                for j2 in range(4):
                    nc.tensor.matmul(
                        psb[32 * j2 : 32 * j2 + 32, :],
                        ars[j2][:, colsel],
                        wos_blk(j2),
                        start=True,
                        stop=True,
                        tile_position=(64 * (j2 % 2), 32 * j2),
                    )
                mb_ps.append(psb)
            psb2 = psum.tile([128, C], FP32, tag="Tq2", name="Tq2")
            for half, colsel in ((0, slice(0, 32)), (1, slice(32, 64))):
                for sub in range(2):
                    rp = 64 * half + 32 * sub
                    nc.tensor.matmul(
                        psb2[rp : rp + 32, :],
                        ars[4 + sub][:, colsel],
                        wos_blk(4 + sub),
                        start=True,
                        stop=True,
                        tile_position=(64 * sub, rp),
                    )

            mb0 = mbp.tile([128, C], BF16, tag="mb0")
            mb1 = mbp.tile([128, C], BF16, tag="mb1")
            mb2 = mbp.tile([128, C], BF16, tag="mb2")
            nc.scalar.activation(mb0[:], mb_ps[0][:], AF.Copy)
            nc.scalar.activation(mb1[:], mb_ps[1][:], AF.Copy)
            nc.scalar.activation(mb2[:], psb2[:], AF.Copy)

            slot = slots[j]
            slot0 = offs[j]
            wins = []
            o = 0
            while o < slot:
                w = min(W, slot - o)
                wins.append((o, w))
                o += w
            stag = outp.tile([128, 3 * slot], BF16, tag="stag")
            for cch in range(3):
                cc = slice(128 * cch, 128 * (cch + 1))
                ops = [
                    psum.tile([128, W], FP32, tag=f"To{wi % 2}", name=f"To{wi % 2}")
                    for wi in range(len(wins))
                ]
                for si, (mb, qmv) in enumerate(((mb0, q0), (mb1, q1), (mb2, q2))):
                    for wi, (o_, w) in enumerate(wins):
                        w0 = slot0 + o_
                        nc.tensor.matmul(
                            ops[wi][:, :w],
                            mb[:, cc],
                            qmv[:, w0 : w0 + w],
                            start=(si == 0),
                            stop=(si == 2),
                        )
                for wi, (o_, w) in enumerate(wins):
                    so = cch * slot + o_
                    if wi % 2 == 0:
                        nc.scalar.activation(
                            stag[:, so : so + w], ops[wi][:, :w], AF.Copy
                        )
                    else:
                        nc.vector.tensor_copy(stag[:, so : so + w], ops[wi][:, :w])
                if is_last:
                    nc.sync.dma_start(
                        out_d[
                            :, 3 * slot0 + cch * slot : 3 * slot0 + (cch + 1) * slot
                        ],
                        stag[:, cch * slot : (cch + 1) * slot],
                    )
            if not is_last:
                nc.sync.dma_start(
                    out_d[:, 3 * slot0 : 3 * slot0 + 3 * slot],
                    stag[:],
                )

        # ------------------------------------------------------------------
        # interleaved schedule
        # ------------------------------------------------------------------
        live = [j for j in range(GPD) if slots[j] > 0]
        kv_ready = {}
        for j in live:
            rc = (offs[j] + slots[j] - 1) // W
            kv_ready.setdefault(rc, []).append(j)
        first_k = live[0] if live else None
        nxt_of = {a: b for a, b in zip(live, live[1:] + [None])}
        for ch in range(NCH):
            emit_chunk(ch)
            if ch == 0 and first_k is not None:
                load_k(first_k)
                load_wo()
            for j in kv_ready.get(ch, []):
                emit_kv(j, nxt_of[j])
            for j in kv_ready.get(ch - 1, []):
                emit_mbout(j, is_last=False)
        for j in kv_ready.get(NCH - 1, []):
            emit_mbout(j, is_last=(j == live[-1]))

    nc.compile()

    _CACHE[key] = (nc, NP)
    return nc, NP


last_exec_time_ns = None
last_results = None


def kernel(x, pos, batch, Wq, bq, Wv, bv, Wo, bo, freqs):
    global last_exec_time_ns
    x = np.asarray(x, dtype=np.float32)
    pos = np.asarray(pos, dtype=np.float32)
    batch = np.asarray(batch).astype(np.int64)
    Wq = np.asarray(Wq, dtype=np.float32)
    bq = np.asarray(bq, dtype=np.float32)
    Wv = np.asarray(Wv, dtype=np.float32)
    bv = np.asarray(bv, dtype=np.float32)
    Wo = np.asarray(Wo, dtype=np.float32)
    bo = np.asarray(bo, dtype=np.float32)
    freqs = np.asarray(freqs, dtype=np.float32)

    counts = np.bincount(batch, minlength=NUM_GRAPHS)
    starts = np.concatenate([[0], np.cumsum(counts)])
    has_bias = bool(np.any(bq) or np.any(bv))

    # All cores share one SPMD program, so slot sizes are per-position.
    # Assign same-size-rank graphs to the same position across cores
    # (largest first) so each position's max-over-cores is tight.
    ranked = np.argsort(counts)[::-1]  # descending
    # gmap[d][lj] = global graph id handled by core d at position lj
    gmap = [[int(ranked[lj * NCORES + d]) for lj in range(GPD)] for d in range(NCORES)]
    slots = []
    for lj in range(GPD):
        mx = max(int(counts[gmap[d][lj]]) for d in range(NCORES))
        slots.append(int(math.ceil(mx / 128.0)) * 128 if mx > 0 else 0)
    slots = tuple(slots)

    nc, NP = _build(slots, has_bias)

    offs = [0]
    for s in slots:
        offs.append(offs[-1] + s)
    NCH = NP // W
    TPSL = [s // 128 for s in slots]
    KEOFF = [0]
    for t in TPSL:
        KEOFF.append(KEOFF[-1] + t * E)

    WqA = Wq[:, _APERM]
    bqA = bq[_APERM]
    bf = ml_dtypes.bfloat16

    wq_p = WqA.reshape(3, 128, E).transpose(1, 0, 2).reshape(128, 3 * E).astype(bf)
    wv_p = Wv.reshape(3, 128, E).transpose(1, 0, 2).reshape(128, 3 * E).astype(bf)
    wos = (Wo * (math.sqrt(2.0) / AVG)).astype(np.float32)
    wo_p = np.zeros((128, 3, C), dtype=bf)
    for bi in range(6):
        r0 = 64 * (bi % 2)
        wo_p[r0 : r0 + 64, bi // 2, :] = wos[64 * bi : 64 * bi + 64, :].astype(bf)
    wo_p = wo_p.reshape(128, 3 * C)

    # phase & trig on host (t = g*16+p, g-major)
    fr = freqs.reshape(NT, SD)
    phase = pos @ fr.T  # [N, 192] float32
    cphase = np.cos(phase)
    sphase = np.sin(phase)
    s2 = 1.0 / math.sqrt(2.0)
    kfull = np.empty((len(x), E), dtype=np.float32)
    k3 = kfull.reshape(len(x), G, D)
    ph3c = cphase.reshape(len(x), G, P)
    ph3s = sphase.reshape(len(x), G, P)
    k3[:, :, 0:P] = (ph3c - ph3s) * s2
    k3[:, :, P:D] = (ph3c + ph3s) * s2

    in_maps = []
    for d in range(NCORES):
        xt = np.zeros((C, NP), dtype=bf)
        cl = np.zeros((NT, NP), dtype=bf)
        sl = np.zeros((NT, NP), dtype=bf)
        kn = np.zeros((128, KEOFF[-1]), dtype=bf)
        xbr = np.zeros((1, NP), dtype=bf)
        for lj in range(GPD):
            gb = gmap[d][lj]
            s, e_, cnt = starts[gb], starts[gb + 1], counts[gb]
            if cnt == 0 or slots[lj] == 0:
                continue
            o = offs[lj]
            xt[:, o : o + cnt] = x[s:e_].T.astype(bf)
            if has_bias:
                xbr[0, o : o + cnt] = 1.0
            cl[:, o : o + cnt] = cphase[s:e_].T.astype(bf)
            sl[:, o : o + cnt] = sphase[s:e_].T.astype(bf)
            kslot = np.zeros((slots[lj], E), dtype=bf)
            kslot[:cnt] = kfull[s:e_].astype(bf)
            kn[:, KEOFF[lj] : KEOFF[lj + 1]] = (
                kslot.reshape(TPSL[lj], 128, E).transpose(1, 0, 2).reshape(128, -1)
            )
        xa = (
            xt.reshape(3, 128, NCH, W)
            .transpose(2, 1, 0, 3)
            .reshape(NCH, 128, 3 * W)
        )
        tr = np.zeros((NCH, 128, 4, W), dtype=bf)
        cl4 = cl.reshape(NT, NCH, W)
        sl4 = sl.reshape(NT, NCH, W)
        tr[:, :, 0, :] = cl4[0:128].transpose(1, 0, 2)
        tr[:, :, 1, :] = sl4[0:128].transpose(1, 0, 2)
        tr[:, 0:64, 2, :] = cl4[128:NT].transpose(1, 0, 2)
        tr[:, 0:64, 3, :] = sl4[128:NT].transpose(1, 0, 2)
        m = {
            "xa": np.ascontiguousarray(xa),
            "tr": tr.reshape(NCH, 128, 4 * W),
            "kn": kn,
            "wq": wq_p,
            "wv": wv_p,
            "wo": wo_p,
        }
        if has_bias:
            m["xb"] = xbr
            m["wqb"] = bqA.astype(bf).reshape(1, E)
            m["wvb"] = bv.astype(bf).reshape(1, E)
        in_maps.append(m)

    want_trace = bool(int(os.environ.get("PLATCONV_TRACE", "0")))
    if want_trace:
        want_trace = _ensure_ntff_hook()
    res = run_bass_kernel_spmd(
        nc,
        in_maps,
        core_ids=list(range(NCORES)),
        trace=want_trace,
    )
    last_exec_time_ns = res.exec_time_ns
    global last_results
    last_results = res

    out = np.zeros((N, C), dtype=np.float32)
    for d in range(NCORES):
        ot = np.asarray(res.results[d]["outt"]).astype(np.float32)
        # ot: [128, 3*NUSED]; graph lj at cols 3*offs[lj], layout [3, slot]
        for lj in range(GPD):
            gb = gmap[d][lj]
            s, e_, cnt = starts[gb], starts[gb + 1], counts[gb]
            if cnt == 0 or slots[lj] == 0:
                continue
            blk = ot[:, 3 * offs[lj] : 3 * offs[lj] + 3 * slots[lj]].reshape(
                128, 3, slots[lj]
            )
            out[s:e_] = blk[:, :, :cnt].transpose(2, 1, 0).reshape(cnt, C)
    out += bo[None, :]
    return out


# revision 23
# speedup vs baseline: 1.0071x; 1.0071x over previous
"""Trainium2 Bass kernel for nn_PlatonicConv (linear-attention GNN message passing).

Math (reference):
  q = rope(x@Wq + bq, phase);  k = rope(ones, phase);  v = x@Wv + bv
  phase[n, g, p] = pos[n, :] . freqs[g, 0, p, :]
  KV_b[g] = (1/AVG) * sum_{n in graph b} k[n,g,:] (x) v[n,g,:]
  out[n]  = concat_g( q'[n,g,:] @ KV_b[g] ) @ Wo + bo

Device formulation (per core, data-parallel over graphs; 8 graphs/core):
  - host precomputes trig (cos/sin of phase, feature-major) and
    k = rope(ones) (pre-tiled per graph, 1/sqrt2 folded into Wo scale).
  - Per graph b:  M_b = stack_g(KV_b[g] @ Wo[g-rows]) : [384, 384]
    out[n] = q'[n] @ M_{b(n)}  (+ bo on host).
  - q'/M_b rows use "A-order" over rope pairs t = g*16+p:
      rows   0:128 = E_t (even dims), t=0..127
      rows 128:256 = O_t (odd  dims), t=0..127
      rows 256:384 = E_t|O_t, t=128..191 (merged 128-part tile q2)
  - graphs get variable 128-aligned slots sized to their node count, so
    no Q/V/rope/out work is spent on padding beyond round-to-128.
  - schedule: per chunk ch emit [chunk ch Q/V/rope] [KV+arena of graphs
    ending at ch] [Mb+out of graphs ending at ch-1]; a full chunk of
    matmuls separates each graph's KV from its Mb consumers so PE never
    waits on the elementwise engines.
  - rope q2 combines run on GpSimd only, with products from Vector
    (concurrent V+GpSimd tensor ops contend ~4x, so GpSimd gets the
    minimal tail of the chain).

Self-contained: hardcodes shapes; shards/pads on host inside kernel().
"""

import math
import os
from contextlib import ExitStack

import ml_dtypes
import numpy as np

import concourse.bacc as bacc_mod
import concourse.bass as bass
import concourse.mybir as mybir
import concourse.tile as tile
from concourse.bass_utils import run_bass_kernel_spmd


def _ensure_ntff_hook():
    """Register the axon NTFF profile hook if the image's antenv lacks it."""
    try:
        import antenv.axon_hooks  # noqa: F401

        return True
    except ImportError:
        pass
    try:
        import sys
        import types

        import antenv
        from trn_agent_boot.trn_boot import _ntff_profile_via_ctypes

        mod = types.ModuleType("antenv.axon_hooks")
        _hook = [None]
        mod.set_axon_ntff_profile_hook = lambda h: _hook.__setitem__(0, h)
        mod.get_axon_ntff_profile_hook = lambda: _hook[0]
        sys.modules["antenv.axon_hooks"] = mod
        antenv.axon_hooks = mod
        mod.set_axon_ntff_profile_hook(
            _ntff_profile_via_ctypes("/opt/axon/libaxon_pjrt.so")
        )
        return True
    except Exception:
        return False


FP32 = mybir.dt.float32
BF16 = mybir.dt.bfloat16
AF = mybir.ActivationFunctionType

N = 32768
C = 384
E = 384
G = 12
D = 32
P = 16
SD = 3
NUM_GRAPHS = 64
NCORES = 8
GPD = NUM_GRAPHS // NCORES  # graphs per device
AVG = float(N) / NUM_GRAPHS  # 512.0
NT = 192  # rope pairs = G*P
W = 512  # streaming window


def _a_order_cols():
    """perm such that A-order column r is original q-dim perm[r]."""
    perm = np.empty(E, dtype=np.int64)
    for r in range(E):
        if r < 128:
            t, odd = r, 0
        elif r < 256:
            t, odd = r - 128, 1
        elif r < 320:
            t, odd = 128 + (r - 256), 0
        else:
            t, odd = 128 + (r - 320), 1
        perm[r] = (t // 16) * 32 + 2 * (t % 16) + odd
    return perm


_APERM = _a_order_cols()

_CACHE = {}


def _build(slots: tuple, has_bias: bool):
    """slots: per-graph 128-aligned node capacities (0 = skip graph)."""
    key = (slots, has_bias)
    if key in _CACHE:
        return _CACHE[key]

    offs = [0]
    for s in slots:
        offs.append(offs[-1] + s)
    NUSED = offs[-1]
    NP = ((NUSED + W - 1) // W) * W
    NTILE = NP // 128
    NCH = NP // W
    TPSL = [s // 128 for s in slots]
    SLOTMAX = max(slots)
    KEOFF = [0]
    for t in TPSL:
        KEOFF.append(KEOFF[-1] + t * E)

    nc = bacc_mod.Bacc()

    nk = 4 if has_bias else 3

    xa_d = nc.declare_dram_parameter("xa", [NCH, 128, 3 * W], BF16, isOutput=False)
    tr_d = nc.declare_dram_parameter("tr", [NCH, 128, 4 * W], BF16, isOutput=False)
    xb_d = None
    if has_bias:
        xb_d = nc.declare_dram_parameter("xb", [1, NP], BF16, isOutput=False)
    kn_d = nc.declare_dram_parameter("kn", [128, KEOFF[-1]], BF16, isOutput=False)
    wq_d = nc.declare_dram_parameter("wq", [128, 3 * E], BF16, isOutput=False)
    wv_d = nc.declare_dram_parameter("wv", [128, 3 * E], BF16, isOutput=False)
    wo_d = nc.declare_dram_parameter("wo", [64, 6 * C], BF16, isOutput=False)
    if has_bias:
        wqb_d = nc.declare_dram_parameter("wqb", [1, E], BF16, isOutput=False)
        wvb_d = nc.declare_dram_parameter("wvb", [1, E], BF16, isOutput=False)
    out_d = nc.declare_dram_parameter("outt", [128, 3 * NUSED], BF16, isOutput=True)

    with ExitStack() as ctx:
        tc = ctx.enter_context(tile.TileContext(nc))

        consts = ctx.enter_context(tc.tile_pool(name="consts", bufs=1))
        xtp = ctx.enter_context(tc.tile_pool(name="xtp", bufs=3))
        qsb = ctx.enter_context(tc.tile_pool(name="qsb", bufs=2))
        big = ctx.enter_context(tc.tile_pool(name="big", bufs=1))
        aren = ctx.enter_context(tc.tile_pool(name="aren", bufs=1))
        mbp = ctx.enter_context(tc.tile_pool(name="mbp", bufs=2))
        outp = ctx.enter_context(tc.tile_pool(name="outp", bufs=2))
        kp = ctx.enter_context(tc.tile_pool(name="kp", bufs=2))
        psum = ctx.enter_context(tc.tile_pool(name="psum", bufs=1, space="PSUM"))

        # ---- constants (weights); wq/wv issued via ACT's DGE so the sync
        # engine can trigger chunk-0 input DMAs concurrently ----
        wq_sb = consts.tile([128, 3, E], BF16, tag="wq")
        nc.scalar.dma_start(wq_sb[:], wq_d[:].rearrange("p (k e) -> p k e", k=3))
        wv_sb = consts.tile([128, 3, E], BF16, tag="wv")
        nc.scalar.dma_start(wv_sb[:], wv_d[:].rearrange("p (k e) -> p k e", k=3))
        wo_sb = consts.tile([64, 6, C], BF16, tag="wo")
        if has_bias:
            wqb = consts.tile([1, E], BF16, tag="wqb")
            nc.scalar.dma_start(wqb[:], wqb_d[:])
            wvb = consts.tile([1, E], BF16, tag="wvb")
            nc.scalar.dma_start(wvb[:], wvb_d[:])

        def load_wo():
            nc.sync.dma_start(wo_sb[:], wo_d[:].rearrange("p (k e) -> p k e", k=6))

        def wq_blk(ki, c0, m):
            if ki < 3:
                return wq_sb[:, ki, c0 : c0 + m]
            return wqb[:, c0 : c0 + m]

        def wv_blk(ki):
            if ki < 3:
                return wv_sb[:, ki, :]
            return wvb[:]

        def wos_blk(bi):  # [64, C] block bi (0..5)
            return wo_sb[:, bi, :]

        # ---- persistent SBUF tensors ----
        q0 = big.tile([128, NP], BF16, tag="q0")
        q1 = big.tile([128, NP], BF16, tag="q1")
        q2 = big.tile([128, NP], BF16, tag="q2")  # rows 0:64 = E2', 64:128 = O2'
        v_sb = big.tile([128, NTILE, E], BF16, tag="v_sb")

        arenas = []
        for s in range(3):
            row_set = []
            for pr in range(6):
                a = aren.tile([64, 64], BF16, tag=f"arena{s}_{pr}")
                nc.vector.memset(a[:], 0.0)
                row_set.append(a)
            arenas.append(row_set)

        # PE warmup: ~3.5us of dummy matmuls so HAM reaches K=8/8 before
        # the first real chunk (its inputs are still in flight on DMA)
        warm = consts.tile([128, 64], BF16, tag="warm")
        nc.vector.memset(warm[:], 0.0)
        wps = psum.tile([64, 64], FP32, tag="To1", name="To1")
        for _ in range(50):
            nc.tensor.matmul(wps[:], warm[:, 0:64], warm[:], start=True, stop=True)

        # k tile prefetch management
        k_tiles = {}

        def load_k(j):
            tps = TPSL[j]
            if tps == 0:
                return
            kt = kp.tile([128, tps, E], BF16, tag="kt")
            nc.sync.dma_start(
                kt[:],
                kn_d[:, KEOFF[j] : KEOFF[j + 1]].rearrange("p (t e) -> p t e", t=tps),
            )
            k_tiles[j] = kt

        # ------------------------------------------------------------------
        # chunk: Q/V projections + rope for nodes [ch*W, ch*W+W)
        # ------------------------------------------------------------------
        def emit_chunk(ch):
            n0 = ch * W
            xa = xtp.tile([128, 3, W], BF16, tag="xa")
            nc.sync.dma_start(xa[:], xa_d[ch, :, :].rearrange("p (k w) -> p k w", k=3))
            tr = xtp.tile([128, 4, W], BF16, tag="tr")
            nc.sync.dma_start(tr[:], tr_d[ch, :, :].rearrange("p (k w) -> p k w", k=4))
            if has_bias:
                xbt = xtp.tile([1, W], BF16, tag="xbt")
                nc.sync.dma_start(xbt[:], xb_d[:, n0 : n0 + W])

            def x_blk(ki):
                if ki < 3:
                    return xa[:, ki, :]
                return xbt[:]

            clf = tr[:, 0, :]
            slf = tr[:, 1, :]
            clh = tr[0:64, 2, :]
            slh = tr[0:64, 3, :]

            # Q projection (A-ordered columns), 3 psum groups of 128
            qps = []
            for g in range(3):
                ps = psum.tile([128, W], FP32, tag=f"Tq{g}", name=f"Tq{g}")
                c0 = 128 * g
                for ki in range(nk):
                    nc.tensor.matmul(
                        ps[:],
                        wq_blk(ki, c0, 128),
                        x_blk(ki),
                        start=(ki == 0),
                        stop=(ki == nk - 1),
                    )
                qps.append(ps)

            # psum -> SBUF casts
            qE0s = qsb.tile([128, W], BF16, tag="qE0s")
            qO0s = qsb.tile([128, W], BF16, tag="qO0s")
            qE2s = qsb.tile([64, W], BF16, tag="qE2s")
            qO2s = qsb.tile([64, W], BF16, tag="qO2s")
            nc.vector.tensor_copy(qE0s[:], qps[0][:])
            nc.vector.tensor_copy(qO0s[:], qps[1][:])
            nc.scalar.activation(qE2s[:], qps[2][0:64, :], AF.Copy)
            nc.scalar.activation(qO2s[:], qps[2][64:128, :], AF.Copy)

            # rope: independent products then combines
            m1 = qsb.tile([128, W], BF16, tag="m1")
            m2 = qsb.tile([128, W], BF16, tag="m2")
            m3 = qsb.tile([128, W], BF16, tag="m3")
            m4 = qsb.tile([128, W], BF16, tag="m4")
            nc.vector.tensor_mul(m1[:], qE0s[:], clf)
            nc.vector.tensor_mul(m2[:], qO0s[:], slf)
            nc.vector.tensor_mul(m3[:], qE0s[:], slf)
            nc.vector.tensor_mul(m4[:], qO0s[:], clf)
            nc.vector.tensor_sub(q0[:, n0 : n0 + W], m1[:], m2[:])
            nc.vector.tensor_add(q1[:, n0 : n0 + W], m3[:], m4[:])

            eng = nc.gpsimd if ch < NCH - 3 else nc.vector
            n1 = qsb.tile([64, W], BF16, tag="n1")
            n2 = qsb.tile([64, W], BF16, tag="n2")
            n3 = qsb.tile([64, W], BF16, tag="n3")
            n4 = qsb.tile([64, W], BF16, tag="n4")
            nc.vector.tensor_mul(n1[:], qE2s[:], clh)
            nc.vector.tensor_mul(n2[:], qO2s[:], slh)
            nc.vector.tensor_mul(n3[:], qE2s[:], slh)
            nc.vector.tensor_mul(n4[:], qO2s[:], clh)
            eng.tensor_sub(q2[0:64, n0 : n0 + W], n1[:], n2[:])
            eng.tensor_add(q2[64:128, n0 : n0 + W], n3[:], n4[:])

            # V per node tile
            for sub in range(W // 128):
                ti = ch * (W // 128) + sub
                f0 = sub * 128
                vps = psum.tile([128, E], FP32, tag="Tv", name="Tv")
                for ki in range(nk):
                    nc.tensor.matmul(
                        vps[:],
                        x_blk(ki)[:, f0 : f0 + 128],
                        wv_blk(ki),
                        start=(ki == 0),
                        stop=(ki == nk - 1),
                    )
                if sub % 2 == 0:
                    nc.vector.tensor_copy(v_sb[:, ti, :], vps[:])
                else:
                    nc.scalar.activation(v_sb[:, ti, :], vps[:], AF.Copy)

        # ------------------------------------------------------------------
        # graph phase 1: KV + arena copies
        # ------------------------------------------------------------------
        def emit_kv(j, nxt):
            if nxt is not None:
                load_k(nxt)
            tps = TPSL[j]
            kt = k_tiles.pop(j)
            t0 = offs[j] // 128

            kvt = psum.tile([128, 3 * 128], FP32, tag=f"Tkv{j % 2}", name=f"Tkv{j % 2}")
            for cchunk in range(3):
                cs = slice(128 * cchunk, 128 * (cchunk + 1))
                for tt in range(tps):
                    nc.tensor.matmul(
                        kvt[:, cs],
                        v_sb[:, t0 + tt, cs],
                        kt[:, tt, cs],
                        start=(tt == 0),
                        stop=(tt == tps - 1),
                    )

            ars = arenas[j % 3]
            for g in range(G):
                cchunk, m = divmod(g, 4)
                pr, par = divmod(g, 2)
                src = kvt[
                    32 * m : 32 * m + 32,
                    128 * cchunk + 32 * m : 128 * cchunk + 32 * m + 32,
                ]
                dst = ars[pr][32 * par : 32 * par + 32, :].rearrange(
                    "e (h s) -> e h s", s=16
                )[:, par::2, :]
                srcr = src.rearrange("e (h s) -> e h s", s=16)
                if g % 2 == 0:
                    nc.vector.tensor_copy(dst, srcr)
                else:
                    nc.scalar.activation(dst, srcr, AF.Copy)

        # ------------------------------------------------------------------
        # graph phase 2: M_b + out matmuls + output DMA
        # ------------------------------------------------------------------
        def emit_mbout(j, is_last):
            ars = arenas[j % 3]
            mb_ps = []
            for cch in range(2):
                psb = psum.tile([128, C], FP32, tag=f"Tq{cch}", name=f"Tq{cch}")
                colsel = slice(0, 32) if cch == 0 else slice(32, 64)
                for j2 in range(4):
                    nc.tensor.matmul(
                        psb[32 * j2 : 32 * j2 + 32, :],
                        ars[j2][:, colsel],
                        wos_blk(j2),
                        start=True,
                        stop=True,
                        tile_position=(0, 32 * j2),
                    )
                mb_ps.append(psb)
            psb2 = psum.tile([128, C], FP32, tag="Tq2", name="Tq2")
            for half, colsel in ((0, slice(0, 32)), (1, slice(32, 64))):
                for sub in range(2):
                    rp = 64 * half + 32 * sub
                    nc.tensor.matmul(
                        psb2[rp : rp + 32, :],
                        ars[4 + sub][:, colsel],
                        wos_blk(4 + sub),
                        start=True,
                        stop=True,
                        tile_position=(0, rp),
                    )

            mb0 = mbp.tile([128, C], BF16, tag="mb0")
            mb1 = mbp.tile([128, C], BF16, tag="mb1")
            mb2 = mbp.tile([128, C], BF16, tag="mb2")
            nc.scalar.activation(mb0[:], mb_ps[0][:], AF.Copy)
            nc.scalar.activation(mb1[:], mb_ps[1][:], AF.Copy)
            nc.scalar.activation(mb2[:], psb2[:], AF.Copy)

            slot = slots[j]
            slot0 = offs[j]
            wins = []
            o = 0
            while o < slot:
                w = min(W, slot - o)
                wins.append((o, w))
                o += w
            stag = outp.tile([128, 3 * slot], BF16, tag="stag")
            for cch in range(3):
                cc = slice(128 * cch, 128 * (cch + 1))
                ops = [
                    psum.tile([128, W], FP32, tag=f"To{wi % 2}", name=f"To{wi % 2}")
                    for wi in range(len(wins))
                ]
                for si, (mb, qmv) in enumerate(((mb0, q0), (mb1, q1), (mb2, q2))):
                    for wi, (o_, w) in enumerate(wins):
                        w0 = slot0 + o_
                        nc.tensor.matmul(
                            ops[wi][:, :w],
                            mb[:, cc],
                            qmv[:, w0 : w0 + w],
                            start=(si == 0),
                            stop=(si == 2),
                        )
                for wi, (o_, w) in enumerate(wins):
                    so = cch * slot + o_
                    if wi % 2 == 0:
                        nc.scalar.activation(
                            stag[:, so : so + w], ops[wi][:, :w], AF.Copy
                        )
                    else:
                        nc.vector.tensor_copy(stag[:, so : so + w], ops[wi][:, :w])
                if is_last:
                    nc.sync.dma_start(
                        out_d[
                            :, 3 * slot0 + cch * slot : 3 * slot0 + (cch + 1) * slot
                        ],
                        stag[:, cch * slot : (cch + 1) * slot],
                    )
            if not is_last:
                nc.sync.dma_start(
                    out_d[:, 3 * slot0 : 3 * slot0 + 3 * slot],
                    stag[:],
                )

        # ------------------------------------------------------------------
        # interleaved schedule
        # ------------------------------------------------------------------
        live = [j for j in range(GPD) if slots[j] > 0]
        kv_ready = {}
        for j in live:
            rc = (offs[j] + slots[j] - 1) // W
            kv_ready.setdefault(rc, []).append(j)
        first_k = live[0] if live else None
        nxt_of = {a: b for a, b in zip(live, live[1:] + [None])}
        for ch in range(NCH):
            emit_chunk(ch)
            if ch == 0 and first_k is not None:
                load_k(first_k)
                load_wo()
            for j in kv_ready.get(ch, []):
                emit_kv(j, nxt_of[j])
            for j in kv_ready.get(ch - 1, []):
                emit_mbout(j, is_last=False)
        for j in kv_ready.get(NCH - 1, []):
            emit_mbout(j, is_last=(j == live[-1]))

    nc.compile()

    _CACHE[key] = (nc, NP)
    return nc, NP


last_exec_time_ns = None
last_results = None


def kernel(x, pos, batch, Wq, bq, Wv, bv, Wo, bo, freqs):
    global last_exec_time_ns
    x = np.asarray(x, dtype=np.float32)
    pos = np.asarray(pos, dtype=np.float32)
    batch = np.asarray(batch).astype(np.int64)
    Wq = np.asarray(Wq, dtype=np.float32)
    bq = np.asarray(bq, dtype=np.float32)
    Wv = np.asarray(Wv, dtype=np.float32)
    bv = np.asarray(bv, dtype=np.float32)
    Wo = np.asarray(Wo, dtype=np.float32)
    bo = np.asarray(bo, dtype=np.float32)
    freqs = np.asarray(freqs, dtype=np.float32)

    counts = np.bincount(batch, minlength=NUM_GRAPHS)
    starts = np.concatenate([[0], np.cumsum(counts)])
    has_bias = bool(np.any(bq) or np.any(bv))

    # All cores share one SPMD program, so slot sizes are per-position.
    # Assign same-size-rank graphs to the same position across cores
    # (largest first) so each position's max-over-cores is tight.
    ranked = np.argsort(counts)[::-1]  # descending
    # gmap[d][lj] = global graph id handled by core d at position lj
    gmap = [[int(ranked[lj * NCORES + d]) for lj in range(GPD)] for d in range(NCORES)]
    slots = []
    for lj in range(GPD):
        mx = max(int(counts[gmap[d][lj]]) for d in range(NCORES))
        slots.append(int(math.ceil(mx / 128.0)) * 128 if mx > 0 else 0)
    slots = tuple(slots)

    nc, NP = _build(slots, has_bias)

    offs = [0]
    for s in slots:
        offs.append(offs[-1] + s)
    NCH = NP // W
    TPSL = [s // 128 for s in slots]
    KEOFF = [0]
    for t in TPSL:
        KEOFF.append(KEOFF[-1] + t * E)

    WqA = Wq[:, _APERM]
    bqA = bq[_APERM]
    bf = ml_dtypes.bfloat16

    wq_p = WqA.reshape(3, 128, E).transpose(1, 0, 2).reshape(128, 3 * E).astype(bf)
    wv_p = Wv.reshape(3, 128, E).transpose(1, 0, 2).reshape(128, 3 * E).astype(bf)
    wos = (Wo * (math.sqrt(2.0) / AVG)).astype(np.float32)
    wo_p = wos.reshape(6, 64, C).transpose(1, 0, 2).reshape(64, 6 * C).astype(bf)

    # phase & trig on host (t = g*16+p, g-major)
    fr = freqs.reshape(NT, SD)
    phase = pos @ fr.T  # [N, 192] float32
    cphase = np.cos(phase)
    sphase = np.sin(phase)
    s2 = 1.0 / math.sqrt(2.0)
    kfull = np.empty((len(x), E), dtype=np.float32)
    k3 = kfull.reshape(len(x), G, D)
    ph3c = cphase.reshape(len(x), G, P)
    ph3s = sphase.reshape(len(x), G, P)
    k3[:, :, 0:P] = (ph3c - ph3s) * s2
    k3[:, :, P:D] = (ph3c + ph3s) * s2

    in_maps = []
    for d in range(NCORES):
        xt = np.zeros((C, NP), dtype=bf)
        cl = np.zeros((NT, NP), dtype=bf)
        sl = np.zeros((NT, NP), dtype=bf)
        kn = np.zeros((128, KEOFF[-1]), dtype=bf)
        xbr = np.zeros((1, NP), dtype=bf)
        for lj in range(GPD):
            gb = gmap[d][lj]
            s, e_, cnt = starts[gb], starts[gb + 1], counts[gb]
            if cnt == 0 or slots[lj] == 0:
                continue
            o = offs[lj]
            xt[:, o : o + cnt] = x[s:e_].T.astype(bf)
            if has_bias:
                xbr[0, o : o + cnt] = 1.0
            cl[:, o : o + cnt] = cphase[s:e_].T.astype(bf)
            sl[:, o : o + cnt] = sphase[s:e_].T.astype(bf)
            kslot = np.zeros((slots[lj], E), dtype=bf)
            kslot[:cnt] = kfull[s:e_].astype(bf)
            kn[:, KEOFF[lj] : KEOFF[lj + 1]] = (
                kslot.reshape(TPSL[lj], 128, E).transpose(1, 0, 2).reshape(128, -1)
            )
        xa = (
            xt.reshape(3, 128, NCH, W)
            .transpose(2, 1, 0, 3)
            .reshape(NCH, 128, 3 * W)
        )
        tr = np.zeros((NCH, 128, 4, W), dtype=bf)
        cl4 = cl.reshape(NT, NCH, W)
        sl4 = sl.reshape(NT, NCH, W)
        tr[:, :, 0, :] = cl4[0:128].transpose(1, 0, 2)
        tr[:, :, 1, :] = sl4[0:128].transpose(1, 0, 2)
        tr[:, 0:64, 2, :] = cl4[128:NT].transpose(1, 0, 2)
        tr[:, 0:64, 3, :] = sl4[128:NT].transpose(1, 0, 2)
        m = {
            "xa": np.ascontiguousarray(xa),
            "tr": tr.reshape(NCH, 128, 4 * W),
            "kn": kn,
            "wq": wq_p,
            "wv": wv_p,
            "wo": wo_p,
        }
        if has_bias:
            m["xb"] = xbr
            m["wqb"] = bqA.astype(bf).reshape(1, E)
            m["wvb"] = bv.astype(bf).reshape(1, E)
        in_maps.append(m)

    want_trace = bool(int(os.environ.get("PLATCONV_TRACE", "0")))
    if want_trace:
        want_trace = _ensure_ntff_hook()
    res = run_bass_kernel_spmd(
        nc,
        in_maps,
        core_ids=list(range(NCORES)),
        trace=want_trace,
    )
    last_exec_time_ns = res.exec_time_ns
    global last_results
    last_results = res

    out = np.zeros((N, C), dtype=np.float32)
    for d in range(NCORES):
        ot = np.asarray(res.results[d]["outt"]).astype(np.float32)
        # ot: [128, 3*NUSED]; graph lj at cols 3*offs[lj], layout [3, slot]
        for lj in range(GPD):
            gb = gmap[d][lj]
            s, e_, cnt = starts[gb], starts[gb + 1], counts[gb]
            if cnt == 0 or slots[lj] == 0:
                continue
            blk = ot[:, 3 * offs[lj] : 3 * offs[lj] + 3 * slots[lj]].reshape(
                128, 3, slots[lj]
            )
            out[s:e_] = blk[:, :, :cnt].transpose(2, 1, 0).reshape(cnt, C)
    out += bo[None, :]
    return out


# revision 24
# speedup vs baseline: 1.0386x; 1.0313x over previous
"""Trainium2 Bass kernel for nn_PlatonicConv (linear-attention GNN message passing).

Math (reference):
  q = rope(x@Wq + bq, phase);  k = rope(ones, phase);  v = x@Wv + bv
  phase[n, g, p] = pos[n, :] . freqs[g, 0, p, :]
  KV_b[g] = (1/AVG) * sum_{n in graph b} k[n,g,:] (x) v[n,g,:]
  out[n]  = concat_g( q'[n,g,:] @ KV_b[g] ) @ Wo + bo

Device formulation (per core, data-parallel over graphs; 8 graphs/core):
  - host precomputes trig (cos/sin of phase, feature-major) and
    k = rope(ones) (pre-tiled per graph, 1/sqrt2 folded into Wo scale).
  - Per graph b:  M_b = stack_g(KV_b[g] @ Wo[g-rows]) : [384, 384]
    out[n] = q'[n] @ M_{b(n)}  (+ bo on host).
  - q'/M_b rows use "A-order" over rope pairs t = g*16+p:
      rows   0:128 = E_t (even dims), t=0..127
      rows 128:256 = O_t (odd  dims), t=0..127
      rows 256:384 = E_t|O_t, t=128..191 (merged 128-part tile q2)
  - graphs get variable 128-aligned slots sized to their node count, so
    no Q/V/rope/out work is spent on padding beyond round-to-128.
  - schedule: per chunk ch emit [chunk ch Q/V/rope] [KV+arena of graphs
    ending at ch] [Mb+out of graphs ending at ch-1]; a full chunk of
    matmuls separates each graph's KV from its Mb consumers so PE never
    waits on the elementwise engines.
  - rope q2 combines run on GpSimd only, with products from Vector
    (concurrent V+GpSimd tensor ops contend ~4x, so GpSimd gets the
    minimal tail of the chain).

Self-contained: hardcodes shapes; shards/pads on host inside kernel().
"""

import math
import os
from contextlib import ExitStack

import ml_dtypes
import numpy as np

import concourse.bacc as bacc_mod
import concourse.bass as bass
import concourse.mybir as mybir
import concourse.tile as tile
from concourse.bass_utils import run_bass_kernel_spmd


def _ensure_ntff_hook():
    """Register the axon NTFF profile hook if the image's antenv lacks it."""
    try:
        import antenv.axon_hooks  # noqa: F401

        return True
    except ImportError:
        pass
    try:
        import sys
        import types

        import antenv
        from trn_agent_boot.trn_boot import _ntff_profile_via_ctypes

        mod = types.ModuleType("antenv.axon_hooks")
        _hook = [None]
        mod.set_axon_ntff_profile_hook = lambda h: _hook.__setitem__(0, h)
        mod.get_axon_ntff_profile_hook = lambda: _hook[0]
        sys.modules["antenv.axon_hooks"] = mod
        antenv.axon_hooks = mod
        mod.set_axon_ntff_profile_hook(
            _ntff_profile_via_ctypes("/opt/axon/libaxon_pjrt.so")
        )
        return True
    except Exception:
        return False


FP32 = mybir.dt.float32
BF16 = mybir.dt.bfloat16
AF = mybir.ActivationFunctionType

N = 32768
C = 384
E = 384
G = 12
D = 32
P = 16
SD = 3
NUM_GRAPHS = 64
NCORES = 8
GPD = NUM_GRAPHS // NCORES  # graphs per device
AVG = float(N) / NUM_GRAPHS  # 512.0
NT = 192  # rope pairs = G*P
W = 512  # streaming window


def _a_order_cols():
    """perm such that A-order column r is original q-dim perm[r]."""
    perm = np.empty(E, dtype=np.int64)
    for r in range(E):
        if r < 128:
            t, odd = r, 0
        elif r < 256:
            t, odd = r - 128, 1
        elif r < 320:
            t, odd = 128 + (r - 256), 0
        else:
            t, odd = 128 + (r - 320), 1
        perm[r] = (t // 16) * 32 + 2 * (t % 16) + odd
    return perm


_APERM = _a_order_cols()

_CACHE = {}


def _build(slots: tuple, has_bias: bool):
    """slots: per-graph 128-aligned node capacities (0 = skip graph)."""
    key = (slots, has_bias)
    if key in _CACHE:
        return _CACHE[key]

    offs = [0]
    for s in slots:
        offs.append(offs[-1] + s)
    NUSED = offs[-1]
    NP = ((NUSED + W - 1) // W) * W
    NTILE = NP // 128
    NCH = NP // W
    TPSL = [s // 128 for s in slots]
    SLOTMAX = max(slots)
    KEOFF = [0]
    for t in TPSL:
        KEOFF.append(KEOFF[-1] + t * E)

    nc = bacc_mod.Bacc()

    nk = 4 if has_bias else 3

    xa_d = nc.declare_dram_parameter("xa", [NCH, 128, 3 * W], BF16, isOutput=False)
    tr_d = nc.declare_dram_parameter("tr", [NCH, 128, 4 * W], BF16, isOutput=False)
    xb_d = None
    if has_bias:
        xb_d = nc.declare_dram_parameter("xb", [1, NP], BF16, isOutput=False)
    kn_d = nc.declare_dram_parameter("kn", [128, KEOFF[-1]], BF16, isOutput=False)
    wq_d = nc.declare_dram_parameter("wq", [128, 3 * E], BF16, isOutput=False)
    wv_d = nc.declare_dram_parameter("wv", [128, 3 * E], BF16, isOutput=False)
    wo_d = nc.declare_dram_parameter("wo", [64, 6 * C], BF16, isOutput=False)
    if has_bias:
        wqb_d = nc.declare_dram_parameter("wqb", [1, E], BF16, isOutput=False)
        wvb_d = nc.declare_dram_parameter("wvb", [1, E], BF16, isOutput=False)
    out_d = nc.declare_dram_parameter("outt", [128, 3 * NUSED], BF16, isOutput=True)

    with ExitStack() as ctx:
        tc = ctx.enter_context(tile.TileContext(nc))

        consts = ctx.enter_context(tc.tile_pool(name="consts", bufs=1))
        xtp = ctx.enter_context(tc.tile_pool(name="xtp", bufs=3))
        qsb = ctx.enter_context(tc.tile_pool(name="qsb", bufs=2))
        big = ctx.enter_context(tc.tile_pool(name="big", bufs=1))
        aren = ctx.enter_context(tc.tile_pool(name="aren", bufs=1))
        mbp = ctx.enter_context(tc.tile_pool(name="mbp", bufs=2))
        outp = ctx.enter_context(tc.tile_pool(name="outp", bufs=2))
        kp = ctx.enter_context(tc.tile_pool(name="kp", bufs=2))
        psum = ctx.enter_context(tc.tile_pool(name="psum", bufs=1, space="PSUM"))

        # ---- constants (weights); wq/wv issued via ACT's DGE so the sync
        # engine can trigger chunk-0 input DMAs concurrently ----
        wq_sb = consts.tile([128, 3, E], BF16, tag="wq")
        nc.scalar.dma_start(wq_sb[:], wq_d[:].rearrange("p (k e) -> p k e", k=3))
        wv_sb = consts.tile([128, 3, E], BF16, tag="wv")
        nc.scalar.dma_start(wv_sb[:], wv_d[:].rearrange("p (k e) -> p k e", k=3))
        wo_sb = consts.tile([64, 6, C], BF16, tag="wo")
        if has_bias:
            wqb = consts.tile([1, E], BF16, tag="wqb")
            nc.scalar.dma_start(wqb[:], wqb_d[:])
            wvb = consts.tile([1, E], BF16, tag="wvb")
            nc.scalar.dma_start(wvb[:], wvb_d[:])

        def load_wo():
            nc.sync.dma_start(wo_sb[:], wo_d[:].rearrange("p (k e) -> p k e", k=6))

        def wq_blk(ki, c0, m):
            if ki < 3:
                return wq_sb[:, ki, c0 : c0 + m]
            return wqb[:, c0 : c0 + m]

        def wv_blk(ki):
            if ki < 3:
                return wv_sb[:, ki, :]
            return wvb[:]

        def wos_blk(bi):  # [64, C] block bi (0..5)
            return wo_sb[:, bi, :]

        # ---- persistent SBUF tensors ----
        q0 = big.tile([128, NP], BF16, tag="q0")
        q1 = big.tile([128, NP], BF16, tag="q1")
        q2 = big.tile([128, NP], BF16, tag="q2")  # rows 0:64 = E2', 64:128 = O2'
        v_sb = big.tile([128, NTILE, E], BF16, tag="v_sb")

        arenas = []
        for s in range(3):
            row_set = []
            for pr in range(6):
                a = aren.tile([64, 64], BF16, tag=f"arena{s}_{pr}")
                nc.vector.memset(a[:], 0.0)
                row_set.append(a)
            arenas.append(row_set)

        # k tile prefetch management
        k_tiles = {}

        def load_k(j):
            tps = TPSL[j]
            if tps == 0:
                return
            kt = kp.tile([128, tps, E], BF16, tag="kt")
            nc.sync.dma_start(
                kt[:],
                kn_d[:, KEOFF[j] : KEOFF[j + 1]].rearrange("p (t e) -> p t e", t=tps),
            )
            k_tiles[j] = kt

        # ------------------------------------------------------------------
        # chunk: Q/V projections + rope for nodes [ch*W, ch*W+W)
        # ------------------------------------------------------------------
        def emit_chunk(ch):
            n0 = ch * W
            xa = xtp.tile([128, 3, W], BF16, tag="xa")
            nc.sync.dma_start(xa[:], xa_d[ch, :, :].rearrange("p (k w) -> p k w", k=3))
            tr = xtp.tile([128, 4, W], BF16, tag="tr")
            nc.sync.dma_start(tr[:], tr_d[ch, :, :].rearrange("p (k w) -> p k w", k=4))
            if has_bias:
                xbt = xtp.tile([1, W], BF16, tag="xbt")
                nc.sync.dma_start(xbt[:], xb_d[:, n0 : n0 + W])

            def x_blk(ki):
                if ki < 3:
                    return xa[:, ki, :]
                return xbt[:]

            clf = tr[:, 0, :]
            slf = tr[:, 1, :]
            clh = tr[0:64, 2, :]
            slh = tr[0:64, 3, :]

            # Q projection (A-ordered columns), 3 psum groups of 128
            qps = []
            for g in range(3):
                ps = psum.tile([128, W], FP32, tag=f"Tq{g}", name=f"Tq{g}")
                c0 = 128 * g
                for ki in range(nk):
                    nc.tensor.matmul(
                        ps[:],
                        wq_blk(ki, c0, 128),
                        x_blk(ki),
                        start=(ki == 0),
                        stop=(ki == nk - 1),
                    )
                qps.append(ps)

            # psum -> SBUF casts
            qE0s = qsb.tile([128, W], BF16, tag="qE0s")
            qO0s = qsb.tile([128, W], BF16, tag="qO0s")
            qE2s = qsb.tile([64, W], BF16, tag="qE2s")
            qO2s = qsb.tile([64, W], BF16, tag="qO2s")
            nc.vector.tensor_copy(qE0s[:], qps[0][:])
            nc.vector.tensor_copy(qO0s[:], qps[1][:])
            nc.scalar.activation(qE2s[:], qps[2][0:64, :], AF.Copy)
            nc.scalar.activation(qO2s[:], qps[2][64:128, :], AF.Copy)

            # rope: independent products then combines
            m1 = qsb.tile([128, W], BF16, tag="m1")
            m2 = qsb.tile([128, W], BF16, tag="m2")
            m3 = qsb.tile([128, W], BF16, tag="m3")
            m4 = qsb.tile([128, W], BF16, tag="m4")
            nc.vector.tensor_mul(m1[:], qE0s[:], clf)
            nc.vector.tensor_mul(m2[:], qO0s[:], slf)
            nc.vector.tensor_mul(m3[:], qE0s[:], slf)
            nc.vector.tensor_mul(m4[:], qO0s[:], clf)
            nc.vector.tensor_sub(q0[:, n0 : n0 + W], m1[:], m2[:])
            nc.vector.tensor_add(q1[:, n0 : n0 + W], m3[:], m4[:])

            eng = nc.gpsimd if ch < NCH - 3 else nc.vector
            n1 = qsb.tile([64, W], BF16, tag="n1")
            n2 = qsb.tile([64, W], BF16, tag="n2")
            n3 = qsb.tile([64, W], BF16, tag="n3")
            n4 = qsb.tile([64, W], BF16, tag="n4")
            nc.vector.tensor_mul(n1[:], qE2s[:], clh)
            nc.vector.tensor_mul(n2[:], qO2s[:], slh)
            nc.vector.tensor_mul(n3[:], qE2s[:], slh)
            nc.vector.tensor_mul(n4[:], qO2s[:], clh)
            eng.tensor_sub(q2[0:64, n0 : n0 + W], n1[:], n2[:])
            eng.tensor_add(q2[64:128, n0 : n0 + W], n3[:], n4[:])

            # V per node tile
            for sub in range(W // 128):
                ti = ch * (W // 128) + sub
                f0 = sub * 128
                vps = psum.tile([128, E], FP32, tag="Tv", name="Tv")
                for ki in range(nk):
                    nc.tensor.matmul(
                        vps[:],
                        x_blk(ki)[:, f0 : f0 + 128],
                        wv_blk(ki),
                        start=(ki == 0),
                        stop=(ki == nk - 1),
                    )
                if sub % 2 == 0:
                    nc.vector.tensor_copy(v_sb[:, ti, :], vps[:])
                else:
                    nc.scalar.activation(v_sb[:, ti, :], vps[:], AF.Copy)

        # ------------------------------------------------------------------
        # graph phase 1: KV + arena copies
        # ------------------------------------------------------------------
        def emit_kv(j, nxt):
            if nxt is not None:
                load_k(nxt)
            tps = TPSL[j]
            kt = k_tiles.pop(j)
            t0 = offs[j] // 128

            kvt = psum.tile([128, 3 * 128], FP32, tag=f"Tkv{j % 2}", name=f"Tkv{j % 2}")
            for cchunk in range(3):
                cs = slice(128 * cchunk, 128 * (cchunk + 1))
                for tt in range(tps):
                    nc.tensor.matmul(
                        kvt[:, cs],
                        v_sb[:, t0 + tt, cs],
                        kt[:, tt, cs],
                        start=(tt == 0),
                        stop=(tt == tps - 1),
                    )

            ars = arenas[j % 3]
            for g in range(G):
                cchunk, m = divmod(g, 4)
                pr, par = divmod(g, 2)
                src = kvt[
                    32 * m : 32 * m + 32,
                    128 * cchunk + 32 * m : 128 * cchunk + 32 * m + 32,
                ]
                dst = ars[pr][32 * par : 32 * par + 32, :].rearrange(
                    "e (h s) -> e h s", s=16
                )[:, par::2, :]
                srcr = src.rearrange("e (h s) -> e h s", s=16)
                if g % 2 == 0:
                    nc.vector.tensor_copy(dst, srcr)
                else:
                    nc.scalar.activation(dst, srcr, AF.Copy)

        # ------------------------------------------------------------------
        # graph phase 2: M_b + out matmuls + output DMA
        # ------------------------------------------------------------------
        def emit_mbout(j, is_last):
            ars = arenas[j % 3]
            mb_ps = []
            for cch in range(2):
                psb = psum.tile([128, C], FP32, tag=f"Tq{cch}", name=f"Tq{cch}")
                colsel = slice(0, 32) if cch == 0 else slice(32, 64)
                for j2 in range(4):
                    nc.tensor.matmul(
                        psb[32 * j2 : 32 * j2 + 32, :],
                        ars[j2][:, colsel],
                        wos_blk(j2),
                        start=True,
                        stop=True,
                        tile_position=(0, 32 * j2),
                    )
                mb_ps.append(psb)
            psb2 = psum.tile([128, C], FP32, tag="Tq2", name="Tq2")
            for half, colsel in ((0, slice(0, 32)), (1, slice(32, 64))):
                for sub in range(2):
                    rp = 64 * half + 32 * sub
                    nc.tensor.matmul(
                        psb2[rp : rp + 32, :],
                        ars[4 + sub][:, colsel],
                        wos_blk(4 + sub),
                        start=True,
                        stop=True,
                        tile_position=(0, rp),
                    )

            mb0 = mbp.tile([128, C], BF16, tag="mb0")
            mb1 = mbp.tile([128, C], BF16, tag="mb1")
            mb2 = mbp.tile([128, C], BF16, tag="mb2")
            nc.scalar.activation(mb0[:], mb_ps[0][:], AF.Copy)
            nc.scalar.activation(mb1[:], mb_ps[1][:], AF.Copy)
            nc.scalar.activation(mb2[:], psb2[:], AF.Copy)

            slot = slots[j]
            slot0 = offs[j]
            wins = []
            o = 0
            while o < slot:
                w = min(W, slot - o)
                wins.append((o, w))
                o += w
            stag = outp.tile([128, 3 * slot], BF16, tag="stag")
            for cch in range(3):
                cc = slice(128 * cch, 128 * (cch + 1))
                ops = [
                    psum.tile([128, W], FP32, tag=f"To{wi % 2}", name=f"To{wi % 2}")
                    for wi in range(len(wins))
                ]
                for si, (mb, qmv) in enumerate(((mb0, q0), (mb1, q1), (mb2, q2))):
                    for wi, (o_, w) in enumerate(wins):
                        w0 = slot0 + o_
                        nc.tensor.matmul(
                            ops[wi][:, :w],
                            mb[:, cc],
                            qmv[:, w0 : w0 + w],
                            start=(si == 0),
                            stop=(si == 2),
                        )
                for wi, (o_, w) in enumerate(wins):
                    so = cch * slot + o_
                    if wi % 2 == 0:
                        nc.scalar.activation(
                            stag[:, so : so + w], ops[wi][:, :w], AF.Copy
                        )
                    else:
                        nc.vector.tensor_copy(stag[:, so : so + w], ops[wi][:, :w])
                if is_last:
                    nc.sync.dma_start(
                        out_d[
                            :, 3 * slot0 + cch * slot : 3 * slot0 + (cch + 1) * slot
                        ],
                        stag[:, cch * slot : (cch + 1) * slot],
                    )
            if not is_last:
                nc.sync.dma_start(
                    out_d[:, 3 * slot0 : 3 * slot0 + 3 * slot],
                    stag[:],
                )

        # ------------------------------------------------------------------
        # interleaved schedule
        # ------------------------------------------------------------------
        live = [j for j in range(GPD) if slots[j] > 0]
        kv_ready = {}
        for j in live:
            rc = (offs[j] + slots[j] - 1) // W
            kv_ready.setdefault(rc, []).append(j)
        first_k = live[0] if live else None
        nxt_of = {a: b for a, b in zip(live, live[1:] + [None])}
        for ch in range(NCH):
            emit_chunk(ch)
            if ch == 0 and first_k is not None:
                load_k(first_k)
                load_wo()
            for j in kv_ready.get(ch, []):
                emit_kv(j, nxt_of[j])
            for j in kv_ready.get(ch - 1, []):
                emit_mbout(j, is_last=False)
        for j in kv_ready.get(NCH - 1, []):
            emit_mbout(j, is_last=(j == live[-1]))

    nc.compile()

    _CACHE[key] = (nc, NP)
    return nc, NP


last_exec_time_ns = None
last_results = None


def kernel(x, pos, batch, Wq, bq, Wv, bv, Wo, bo, freqs):
    global last_exec_time_ns
    x = np.asarray(x, dtype=np.float32)
    pos = np.asarray(pos, dtype=np.float32)
    batch = np.asarray(batch).astype(np.int64)
    Wq = np.asarray(Wq, dtype=np.float32)
    bq = np.asarray(bq, dtype=np.float32)
    Wv = np.asarray(Wv, dtype=np.float32)
    bv = np.asarray(bv, dtype=np.float32)
    Wo = np.asarray(Wo, dtype=np.float32)
    bo = np.asarray(bo, dtype=np.float32)
    freqs = np.asarray(freqs, dtype=np.float32)

    counts = np.bincount(batch, minlength=NUM_GRAPHS)
    starts = np.concatenate([[0], np.cumsum(counts)])
    has_bias = bool(np.any(bq) or np.any(bv))

    # All cores share one SPMD program, so slot sizes are per-position.
    # Assign same-size-rank graphs to the same position across cores
    # (largest first) so each position's max-over-cores is tight.
    ranked = np.argsort(counts)[::-1]  # descending
    # gmap[d][lj] = global graph id handled by core d at position lj
    gmap = [[int(ranked[lj * NCORES + d]) for lj in range(GPD)] for d in range(NCORES)]
    slots = []
    for lj in range(GPD):
        mx = max(int(counts[gmap[d][lj]]) for d in range(NCORES))
        slots.append(int(math.ceil(mx / 128.0)) * 128 if mx > 0 else 0)
    slots = tuple(slots)

    nc, NP = _build(slots, has_bias)

    offs = [0]
    for s in slots:
        offs.append(offs[-1] + s)
    NCH = NP // W
    TPSL = [s // 128 for s in slots]
    KEOFF = [0]
    for t in TPSL:
        KEOFF.append(KEOFF[-1] + t * E)

    WqA = Wq[:, _APERM]
    bqA = bq[_APERM]
    bf = ml_dtypes.bfloat16

    wq_p = WqA.reshape(3, 128, E).transpose(1, 0, 2).reshape(128, 3 * E).astype(bf)
    wv_p = Wv.reshape(3, 128, E).transpose(1, 0, 2).reshape(128, 3 * E).astype(bf)
    wos = (Wo * (math.sqrt(2.0) / AVG)).astype(np.float32)
    wo_p = wos.reshape(6, 64, C).transpose(1, 0, 2).reshape(64, 6 * C).astype(bf)

    # phase & trig on host (t = g*16+p, g-major)
    fr = freqs.reshape(NT, SD)
    phase = pos @ fr.T  # [N, 192] float32
    cphase = np.cos(phase)
    sphase = np.sin(phase)
    s2 = 1.0 / math.sqrt(2.0)
    kfull = np.empty((len(x), E), dtype=np.float32)
    k3 = kfull.reshape(len(x), G, D)
    ph3c = cphase.reshape(len(x), G, P)
    ph3s = sphase.reshape(len(x), G, P)
    k3[:, :, 0:P] = (ph3c - ph3s) * s2
    k3[:, :, P:D] = (ph3c + ph3s) * s2

    in_maps = []
    for d in range(NCORES):
        xt = np.zeros((C, NP), dtype=bf)
        cl = np.zeros((NT, NP), dtype=bf)
        sl = np.zeros((NT, NP), dtype=bf)
        kn = np.zeros((128, KEOFF[-1]), dtype=bf)
        xbr = np.zeros((1, NP), dtype=bf)
        for lj in range(GPD):
            gb = gmap[d][lj]
            s, e_, cnt = starts[gb], starts[gb + 1], counts[gb]
            if cnt == 0 or slots[lj] == 0:
                continue
            o = offs[lj]
            xt[:, o : o + cnt] = x[s:e_].T.astype(bf)
            if has_bias:
                xbr[0, o : o + cnt] = 1.0
            cl[:, o : o + cnt] = cphase[s:e_].T.astype(bf)
            sl[:, o : o + cnt] = sphase[s:e_].T.astype(bf)
            kslot = np.zeros((slots[lj], E), dtype=bf)
            kslot[:cnt] = kfull[s:e_].astype(bf)
            kn[:, KEOFF[lj] : KEOFF[lj + 1]] = (
                kslot.reshape(TPSL[lj], 128, E).transpose(1, 0, 2).reshape(128, -1)
            )
        xa = (
            xt.reshape(3, 128, NCH, W)
            .transpose(2, 1, 0, 3)
            .reshape(NCH, 128, 3 * W)
        )
        tr = np.zeros((NCH, 128, 4, W), dtype=bf)
        cl4 = cl.reshape(NT, NCH, W)
        sl4 = sl.reshape(NT, NCH, W)
        tr[:, :, 0, :] = cl4[0:128].transpose(1, 0, 2)
        tr[:, :, 1, :] = sl4[0:128].transpose(1, 0, 2)
        tr[:, 0:64, 2, :] = cl4[128:NT].transpose(1, 0, 2)
        tr[:, 0:64, 3, :] = sl4[128:NT].transpose(1, 0, 2)
        m = {
            "xa": np.ascontiguousarray(xa),
            "tr": tr.reshape(NCH, 128, 4 * W),
            "kn": kn,
            "wq": wq_p,
            "wv": wv_p,
            "wo": wo_p,
        }
        if has_bias:
            m["xb"] = xbr
            m["wqb"] = bqA.astype(bf).reshape(1, E)
            m["wvb"] = bv.astype(bf).reshape(1, E)
        in_maps.append(m)

    want_trace = bool(int(os.environ.get("PLATCONV_TRACE", "0")))
    if want_trace:
        want_trace = _ensure_ntff_hook()
    res = run_bass_kernel_spmd(
        nc,
        in_maps,
        core_ids=list(range(NCORES)),
        trace=want_trace,
    )
    last_exec_time_ns = res.exec_time_ns
    global last_results
    last_results = res

    out = np.zeros((N, C), dtype=np.float32)
    for d in range(NCORES):
        ot = np.asarray(res.results[d]["outt"]).astype(np.float32)
        # ot: [128, 3*NUSED]; graph lj at cols 3*offs[lj], layout [3, slot]
        for lj in range(GPD):
            gb = gmap[d][lj]
            s, e_, cnt = starts[gb], starts[gb + 1], counts[gb]
            if cnt == 0 or slots[lj] == 0:
                continue
            blk = ot[:, 3 * offs[lj] : 3 * offs[lj] + 3 * slots[lj]].reshape(
                128, 3, slots[lj]
            )
            out[s:e_] = blk[:, :, :cnt].transpose(2, 1, 0).reshape(cnt, C)
    out += bo[None, :]
    return out


# revision 25
# speedup vs baseline: 1.0826x; 1.0424x over previous
"""Trainium2 Bass kernel for nn_PlatonicConv (linear-attention GNN message passing).

Math (reference):
  q = rope(x@Wq + bq, phase);  k = rope(ones, phase);  v = x@Wv + bv
  phase[n, g, p] = pos[n, :] . freqs[g, 0, p, :]
  KV_b[g] = (1/AVG) * sum_{n in graph b} k[n,g,:] (x) v[n,g,:]
  out[n]  = concat_g( q'[n,g,:] @ KV_b[g] ) @ Wo + bo

Device formulation (per core, data-parallel over graphs; 8 graphs/core):
  - host precomputes trig (cos/sin of phase, feature-major) and
    k = rope(ones) (pre-tiled per graph, 1/sqrt2 folded into Wo scale).
  - Per graph b:  M_b = stack_g(KV_b[g] @ Wo[g-rows]) : [384, 384]
    out[n] = q'[n] @ M_{b(n)}  (+ bo on host).
  - q'/M_b rows use "A-order" over rope pairs t = g*16+p:
      rows   0:128 = E_t (even dims), t=0..127
      rows 128:256 = O_t (odd  dims), t=0..127
      rows 256:384 = E_t|O_t, t=128..191 (merged 128-part tile q2)
  - graphs get variable 128-aligned slots sized to their node count, so
    no Q/V/rope/out work is spent on padding beyond round-to-128.
  - schedule: per chunk ch emit [chunk ch Q/V/rope] [KV+arena of graphs
    ending at ch] [Mb+out of graphs ending at ch-1]; a full chunk of
    matmuls separates each graph's KV from its Mb consumers so PE never
    waits on the elementwise engines.
  - rope q2 combines run on GpSimd only, with products from Vector
    (concurrent V+GpSimd tensor ops contend ~4x, so GpSimd gets the
    minimal tail of the chain).

Self-contained: hardcodes shapes; shards/pads on host inside kernel().
"""

import math
import os
from contextlib import ExitStack

import ml_dtypes
import numpy as np

import concourse.bacc as bacc_mod
import concourse.bass as bass
import concourse.mybir as mybir
import concourse.tile as tile
from concourse.bass_utils import run_bass_kernel_spmd


def _ensure_ntff_hook():
    """Register the axon NTFF profile hook if the image's antenv lacks it."""
    try:
        import antenv.axon_hooks  # noqa: F401

        return True
    except ImportError:
        pass
    try:
        import sys
        import types

        import antenv
        from trn_agent_boot.trn_boot import _ntff_profile_via_ctypes

        mod = types.ModuleType("antenv.axon_hooks")
        _hook = [None]
        mod.set_axon_ntff_profile_hook = lambda h: _hook.__setitem__(0, h)
        mod.get_axon_ntff_profile_hook = lambda: _hook[0]
        sys.modules["antenv.axon_hooks"] = mod
        antenv.axon_hooks = mod
        mod.set_axon_ntff_profile_hook(
            _ntff_profile_via_ctypes("/opt/axon/libaxon_pjrt.so")
        )
        return True
    except Exception:
        return False


FP32 = mybir.dt.float32
BF16 = mybir.dt.bfloat16
AF = mybir.ActivationFunctionType

N = 32768
C = 384
E = 384
G = 12
D = 32
P = 16
SD = 3
NUM_GRAPHS = 64
NCORES = 8
GPD = NUM_GRAPHS // NCORES  # graphs per device
AVG = float(N) / NUM_GRAPHS  # 512.0
NT = 192  # rope pairs = G*P
W = 512  # streaming window


def _a_order_cols():
    """perm such that A-order column r is original q-dim perm[r]."""
    perm = np.empty(E, dtype=np.int64)
    for r in range(E):
        if r < 128:
            t, odd = r, 0
        elif r < 256:
            t, odd = r - 128, 1
        elif r < 320:
            t, odd = 128 + (r - 256), 0
        else:
            t, odd = 128 + (r - 320), 1
        perm[r] = (t // 16) * 32 + 2 * (t % 16) + odd
    return perm


_APERM = _a_order_cols()

_CACHE = {}


def _build(slots: tuple, has_bias: bool):
    """slots: per-graph 128-aligned node capacities (0 = skip graph)."""
    key = (slots, has_bias)
    if key in _CACHE:
        return _CACHE[key]

    offs = [0]
    for s in slots:
        offs.append(offs[-1] + s)
    NUSED = offs[-1]
    NP = ((NUSED + W - 1) // W) * W
    NTILE = NP // 128
    NCH = NP // W
    TPSL = [s // 128 for s in slots]
    SLOTMAX = max(slots)
    KEOFF = [0]
    for t in TPSL:
        KEOFF.append(KEOFF[-1] + t * E)

    nc = bacc_mod.Bacc()

    nk = 4 if has_bias else 3

    xa_d = nc.declare_dram_parameter("xa", [NCH, 128, 3 * W], BF16, isOutput=False)
    tr_d = nc.declare_dram_parameter("tr", [NCH, 128, 4 * W], BF16, isOutput=False)
    xb_d = None
    if has_bias:
        xb_d = nc.declare_dram_parameter("xb", [1, NP], BF16, isOutput=False)
    kn_d = nc.declare_dram_parameter("kn", [128, KEOFF[-1]], BF16, isOutput=False)
    wq_d = nc.declare_dram_parameter("wq", [128, 3 * E], BF16, isOutput=False)
    wv_d = nc.declare_dram_parameter("wv", [128, 3 * E], BF16, isOutput=False)
    wo_d = nc.declare_dram_parameter("wo", [64, 6 * C], BF16, isOutput=False)
    if has_bias:
        wqb_d = nc.declare_dram_parameter("wqb", [1, E], BF16, isOutput=False)
        wvb_d = nc.declare_dram_parameter("wvb", [1, E], BF16, isOutput=False)
    out_d = nc.declare_dram_parameter("outt", [128, 3 * NUSED], BF16, isOutput=True)

    with ExitStack() as ctx:
        tc = ctx.enter_context(tile.TileContext(nc))

        consts = ctx.enter_context(tc.tile_pool(name="consts", bufs=1))
        xtp = ctx.enter_context(tc.tile_pool(name="xtp", bufs=3))
        qsb = ctx.enter_context(tc.tile_pool(name="qsb", bufs=3))
        big = ctx.enter_context(tc.tile_pool(name="big", bufs=1))
        aren = ctx.enter_context(tc.tile_pool(name="aren", bufs=1))
        mbp = ctx.enter_context(tc.tile_pool(name="mbp", bufs=3))
        outp = ctx.enter_context(tc.tile_pool(name="outp", bufs=3))
        kp = ctx.enter_context(tc.tile_pool(name="kp", bufs=2))
        psum = ctx.enter_context(tc.tile_pool(name="psum", bufs=1, space="PSUM"))

        # ---- constants (weights); wq/wv issued via ACT's DGE so the sync
        # engine can trigger chunk-0 input DMAs concurrently ----
        wq_sb = consts.tile([128, 3, E], BF16, tag="wq")
        nc.scalar.dma_start(wq_sb[:], wq_d[:].rearrange("p (k e) -> p k e", k=3))
        wv_sb = consts.tile([128, 3, E], BF16, tag="wv")
        nc.scalar.dma_start(wv_sb[:], wv_d[:].rearrange("p (k e) -> p k e", k=3))
        wo_sb = consts.tile([64, 6, C], BF16, tag="wo")
        if has_bias:
            wqb = consts.tile([1, E], BF16, tag="wqb")
            nc.scalar.dma_start(wqb[:], wqb_d[:])
            wvb = consts.tile([1, E], BF16, tag="wvb")
            nc.scalar.dma_start(wvb[:], wvb_d[:])

        def load_wo():
            nc.sync.dma_start(wo_sb[:], wo_d[:].rearrange("p (k e) -> p k e", k=6))

        def wq_blk(ki, c0, m):
            if ki < 3:
                return wq_sb[:, ki, c0 : c0 + m]
            return wqb[:, c0 : c0 + m]

        def wv_blk(ki):
            if ki < 3:
                return wv_sb[:, ki, :]
            return wvb[:]

        def wos_blk(bi):  # [64, C] block bi (0..5)
            return wo_sb[:, bi, :]

        # ---- persistent SBUF tensors ----
        q0 = big.tile([128, NP], BF16, tag="q0")
        q1 = big.tile([128, NP], BF16, tag="q1")
        q2 = big.tile([128, NP], BF16, tag="q2")  # rows 0:64 = E2', 64:128 = O2'
        v_sb = big.tile([128, NTILE, E], BF16, tag="v_sb")

        arenas = []
        for s in range(3):
            row_set = []
            for pr in range(6):
                a = aren.tile([64, 64], BF16, tag=f"arena{s}_{pr}")
                nc.vector.memset(a[:], 0.0)
                row_set.append(a)
            arenas.append(row_set)

        # k tile prefetch management
        k_tiles = {}

        def load_k(j):
            tps = TPSL[j]
            if tps == 0:
                return
            kt = kp.tile([128, tps, E], BF16, tag="kt")
            nc.sync.dma_start(
                kt[:],
                kn_d[:, KEOFF[j] : KEOFF[j + 1]].rearrange("p (t e) -> p t e", t=tps),
            )
            k_tiles[j] = kt

        # ------------------------------------------------------------------
        # chunk: Q/V projections + rope for nodes [ch*W, ch*W+W)
        # ------------------------------------------------------------------
        def emit_chunk(ch):
            n0 = ch * W
            xa = xtp.tile([128, 3, W], BF16, tag="xa")
            nc.sync.dma_start(xa[:], xa_d[ch, :, :].rearrange("p (k w) -> p k w", k=3))
            tr = xtp.tile([128, 4, W], BF16, tag="tr")
            nc.sync.dma_start(tr[:], tr_d[ch, :, :].rearrange("p (k w) -> p k w", k=4))
            if has_bias:
                xbt = xtp.tile([1, W], BF16, tag="xbt")
                nc.sync.dma_start(xbt[:], xb_d[:, n0 : n0 + W])

            def x_blk(ki):
                if ki < 3:
                    return xa[:, ki, :]
                return xbt[:]

            clf = tr[:, 0, :]
            slf = tr[:, 1, :]
            clh = tr[0:64, 2, :]
            slh = tr[0:64, 3, :]

            # Q projection (A-ordered columns), 3 psum groups of 128
            qps = []
            for g in range(3):
                ps = psum.tile([128, W], FP32, tag=f"Tq{g}", name=f"Tq{g}")
                c0 = 128 * g
                for ki in range(nk):
                    nc.tensor.matmul(
                        ps[:],
                        wq_blk(ki, c0, 128),
                        x_blk(ki),
                        start=(ki == 0),
                        stop=(ki == nk - 1),
                    )
                qps.append(ps)

            # psum -> SBUF casts
            qE0s = qsb.tile([128, W], BF16, tag="qE0s")
            qO0s = qsb.tile([128, W], BF16, tag="qO0s")
            qE2s = qsb.tile([64, W], BF16, tag="qE2s")
            qO2s = qsb.tile([64, W], BF16, tag="qO2s")
            nc.vector.tensor_copy(qE0s[:], qps[0][:])
            nc.vector.tensor_copy(qO0s[:], qps[1][:])
            nc.scalar.activation(qE2s[:], qps[2][0:64, :], AF.Copy)
            nc.scalar.activation(qO2s[:], qps[2][64:128, :], AF.Copy)

            # rope: independent products then combines
            m1 = qsb.tile([128, W], BF16, tag="m1")
            m2 = qsb.tile([128, W], BF16, tag="m2")
            m3 = qsb.tile([128, W], BF16, tag="m3")
            m4 = qsb.tile([128, W], BF16, tag="m4")
            nc.vector.tensor_mul(m1[:], qE0s[:], clf)
            nc.vector.tensor_mul(m2[:], qO0s[:], slf)
            nc.vector.tensor_mul(m3[:], qE0s[:], slf)
            nc.vector.tensor_mul(m4[:], qO0s[:], clf)
            nc.vector.tensor_sub(q0[:, n0 : n0 + W], m1[:], m2[:])
            nc.vector.tensor_add(q1[:, n0 : n0 + W], m3[:], m4[:])

            eng = nc.gpsimd if ch < NCH - 3 else nc.vector
            n1 = qsb.tile([64, W], BF16, tag="n1")
            n2 = qsb.tile([64, W], BF16, tag="n2")
            n3 = qsb.tile([64, W], BF16, tag="n3")
            n4 = qsb.tile([64, W], BF16, tag="n4")
            nc.vector.tensor_mul(n1[:], qE2s[:], clh)
            nc.vector.tensor_mul(n2[:], qO2s[:], slh)
            nc.vector.tensor_mul(n3[:], qE2s[:], slh)
            nc.vector.tensor_mul(n4[:], qO2s[:], clh)
            eng.tensor_sub(q2[0:64, n0 : n0 + W], n1[:], n2[:])
            eng.tensor_add(q2[64:128, n0 : n0 + W], n3[:], n4[:])

            # V per node tile
            for sub in range(W // 128):
                ti = ch * (W // 128) + sub
                f0 = sub * 128
                vt = f"Tv{sub % 2}"
                vps = psum.tile([128, E], FP32, tag=vt, name=vt)
                for ki in range(nk):
                    nc.tensor.matmul(
                        vps[:],
                        x_blk(ki)[:, f0 : f0 + 128],
                        wv_blk(ki),
                        start=(ki == 0),
                        stop=(ki == nk - 1),
                    )
                if sub % 2 == 0:
                    nc.vector.tensor_copy(v_sb[:, ti, :], vps[:])
                else:
                    nc.scalar.activation(v_sb[:, ti, :], vps[:], AF.Copy)

        # ------------------------------------------------------------------
        # graph phase 1: KV + arena copies
        # ------------------------------------------------------------------
        def emit_kv(j, nxt):
            if nxt is not None:
                load_k(nxt)
            tps = TPSL[j]
            kt = k_tiles.pop(j)
            t0 = offs[j] // 128

            kvt = psum.tile([128, 3 * 128], FP32, tag="Tkv", name="Tkv")
            for cchunk in range(3):
                cs = slice(128 * cchunk, 128 * (cchunk + 1))
                for tt in range(tps):
                    nc.tensor.matmul(
                        kvt[:, cs],
                        v_sb[:, t0 + tt, cs],
                        kt[:, tt, cs],
                        start=(tt == 0),
                        stop=(tt == tps - 1),
                    )

            ars = arenas[j % 3]
            for g in range(G):
                cchunk, m = divmod(g, 4)
                pr, par = divmod(g, 2)
                src = kvt[
                    32 * m : 32 * m + 32,
                    128 * cchunk + 32 * m : 128 * cchunk + 32 * m + 32,
                ]
                dst = ars[pr][32 * par : 32 * par + 32, :].rearrange(
                    "e (h s) -> e h s", s=16
                )[:, par::2, :]
                srcr = src.rearrange("e (h s) -> e h s", s=16)
                if g % 2 == 0:
                    nc.vector.tensor_copy(dst, srcr)
                else:
                    nc.scalar.activation(dst, srcr, AF.Copy)

        # ------------------------------------------------------------------
        # graph phase 2: M_b + out matmuls + output DMA
        # ------------------------------------------------------------------
        def emit_mbout(j, is_last):
            ars = arenas[j % 3]
            mb_ps = []
            for cch in range(2):
                psb = psum.tile([128, C], FP32, tag=f"Tq{cch}", name=f"Tq{cch}")
                colsel = slice(0, 32) if cch == 0 else slice(32, 64)
                for j2 in range(4):
                    nc.tensor.matmul(
                        psb[32 * j2 : 32 * j2 + 32, :],
                        ars[j2][:, colsel],
                        wos_blk(j2),
                        start=True,
                        stop=True,
                        tile_position=(0, 32 * j2),
                    )
                mb_ps.append(psb)
            psb2 = psum.tile([128, C], FP32, tag="Tq2", name="Tq2")
            for half, colsel in ((0, slice(0, 32)), (1, slice(32, 64))):
                for sub in range(2):
                    rp = 64 * half + 32 * sub
                    nc.tensor.matmul(
                        psb2[rp : rp + 32, :],
                        ars[4 + sub][:, colsel],
                        wos_blk(4 + sub),
                        start=True,
                        stop=True,
                        tile_position=(0, rp),
                    )

            mb0 = mbp.tile([128, C], BF16, tag="mb0")
            mb1 = mbp.tile([128, C], BF16, tag="mb1")
            mb2 = mbp.tile([128, C], BF16, tag="mb2")
            nc.scalar.activation(mb0[:], mb_ps[0][:], AF.Copy)
            nc.scalar.activation(mb1[:], mb_ps[1][:], AF.Copy)
            nc.scalar.activation(mb2[:], psb2[:], AF.Copy)

            slot = slots[j]
            slot0 = offs[j]
            wins = []
            o = 0
            while o < slot:
                w = min(W, slot - o)
                wins.append((o, w))
                o += w
            stag = outp.tile([128, 3 * slot], BF16, tag="stag")
            for cch in range(3):
                cc = slice(128 * cch, 128 * (cch + 1))
                ops = [
                    psum.tile([128, W], FP32, tag=f"To{wi % 2}", name=f"To{wi % 2}")
                    for wi in range(len(wins))
                ]
                for si, (mb, qmv) in enumerate(((mb0, q0), (mb1, q1), (mb2, q2))):
                    for wi, (o_, w) in enumerate(wins):
                        w0 = slot0 + o_
                        nc.tensor.matmul(
                            ops[wi][:, :w],
                            mb[:, cc],
                            qmv[:, w0 : w0 + w],
                            start=(si == 0),
                            stop=(si == 2),
                        )
                for wi, (o_, w) in enumerate(wins):
                    so = cch * slot + o_
                    if wi % 2 == 0:
                        nc.scalar.activation(
                            stag[:, so : so + w], ops[wi][:, :w], AF.Copy
                        )
                    else:
                        nc.vector.tensor_copy(stag[:, so : so + w], ops[wi][:, :w])
                if is_last:
                    nc.sync.dma_start(
                        out_d[
                            :, 3 * slot0 + cch * slot : 3 * slot0 + (cch + 1) * slot
                        ],
                        stag[:, cch * slot : (cch + 1) * slot],
                    )
            if not is_last:
                nc.sync.dma_start(
                    out_d[:, 3 * slot0 : 3 * slot0 + 3 * slot],
                    stag[:],
                )

        # ------------------------------------------------------------------
        # interleaved schedule
        # ------------------------------------------------------------------
        live = [j for j in range(GPD) if slots[j] > 0]
        kv_ready = {}
        for j in live:
            rc = (offs[j] + slots[j] - 1) // W
            kv_ready.setdefault(rc, []).append(j)
        first_k = live[0] if live else None
        nxt_of = {a: b for a, b in zip(live, live[1:] + [None])}
        for ch in range(NCH):
            emit_chunk(ch)
            if ch == 0 and first_k is not None:
                load_k(first_k)
                load_wo()
            for j in kv_ready.get(ch, []):
                emit_kv(j, nxt_of[j])
            for j in kv_ready.get(ch - 1, []):
                emit_mbout(j, is_last=False)
        for j in kv_ready.get(NCH - 1, []):
            emit_mbout(j, is_last=(j == live[-1]))

    nc.compile()

    _CACHE[key] = (nc, NP)
    return nc, NP


last_exec_time_ns = None
last_results = None


def kernel(x, pos, batch, Wq, bq, Wv, bv, Wo, bo, freqs):
    global last_exec_time_ns
    x = np.asarray(x, dtype=np.float32)
    pos = np.asarray(pos, dtype=np.float32)
    batch = np.asarray(batch).astype(np.int64)
    Wq = np.asarray(Wq, dtype=np.float32)
    bq = np.asarray(bq, dtype=np.float32)
    Wv = np.asarray(Wv, dtype=np.float32)
    bv = np.asarray(bv, dtype=np.float32)
    Wo = np.asarray(Wo, dtype=np.float32)
    bo = np.asarray(bo, dtype=np.float32)
    freqs = np.asarray(freqs, dtype=np.float32)

    counts = np.bincount(batch, minlength=NUM_GRAPHS)
    starts = np.concatenate([[0], np.cumsum(counts)])
    has_bias = bool(np.any(bq) or np.any(bv))

    # All cores share one SPMD program, so slot sizes are per-position.
    # Assign same-size-rank graphs to the same position across cores
    # (largest first) so each position's max-over-cores is tight.
    ranked = np.argsort(counts)[::-1]  # descending
    # gmap[d][lj] = global graph id handled by core d at position lj
    gmap = [[int(ranked[lj * NCORES + d]) for lj in range(GPD)] for d in range(NCORES)]
    slots = []
    for lj in range(GPD):
        mx = max(int(counts[gmap[d][lj]]) for d in range(NCORES))
        slots.append(int(math.ceil(mx / 128.0)) * 128 if mx > 0 else 0)
    slots = tuple(slots)

    nc, NP = _build(slots, has_bias)

    offs = [0]
    for s in slots:
        offs.append(offs[-1] + s)
    NCH = NP // W
    TPSL = [s // 128 for s in slots]
    KEOFF = [0]
    for t in TPSL:
        KEOFF.append(KEOFF[-1] + t * E)

    WqA = Wq[:, _APERM]
    bqA = bq[_APERM]
    bf = ml_dtypes.bfloat16

    wq_p = WqA.reshape(3, 128, E).transpose(1, 0, 2).reshape(128, 3 * E).astype(bf)
    wv_p = Wv.reshape(3, 128, E).transpose(1, 0, 2).reshape(128, 3 * E).astype(bf)
    wos = (Wo * (math.sqrt(2.0) / AVG)).astype(np.float32)
    wo_p = wos.reshape(6, 64, C).transpose(1, 0, 2).reshape(64, 6 * C).astype(bf)

    # phase & trig on host (t = g*16+p, g-major)
    fr = freqs.reshape(NT, SD)
    phase = pos @ fr.T  # [N, 192] float32
    cphase = np.cos(phase)
    sphase = np.sin(phase)
    s2 = 1.0 / math.sqrt(2.0)
    kfull = np.empty((len(x), E), dtype=np.float32)
    k3 = kfull.reshape(len(x), G, D)
    ph3c = cphase.reshape(len(x), G, P)
    ph3s = sphase.reshape(len(x), G, P)
    k3[:, :, 0:P] = (ph3c - ph3s) * s2
    k3[:, :, P:D] = (ph3c + ph3s) * s2

    in_maps = []
    for d in range(NCORES):
        xt = np.zeros((C, NP), dtype=bf)
        cl = np.zeros((NT, NP), dtype=bf)
        sl = np.zeros((NT, NP), dtype=bf)
        kn = np.zeros((128, KEOFF[-1]), dtype=bf)
        xbr = np.zeros((1, NP), dtype=bf)
        for lj in range(GPD):
            gb = gmap[d][lj]
            s, e_, cnt = starts[gb], starts[gb + 1], counts[gb]
            if cnt == 0 or slots[lj] == 0:
                continue
            o = offs[lj]
            xt[:, o : o + cnt] = x[s:e_].T.astype(bf)
            if has_bias:
                xbr[0, o : o + cnt] = 1.0
            cl[:, o : o + cnt] = cphase[s:e_].T.astype(bf)
            sl[:, o : o + cnt] = sphase[s:e_].T.astype(bf)
            kslot = np.zeros((slots[lj], E), dtype=bf)
            kslot[:cnt] = kfull[s:e_].astype(bf)
            kn[:, KEOFF[lj] : KEOFF[lj + 1]] = (
                kslot.reshape(TPSL[lj], 128, E).transpose(1, 0, 2).reshape(128, -1)
            )
        xa = (
            xt.reshape(3, 128, NCH, W)
            .transpose(2, 1, 0, 3)
            .reshape(NCH, 128, 3 * W)
        )
        tr = np.zeros((NCH, 128, 4, W), dtype=bf)
        cl4 = cl.reshape(NT, NCH, W)
        sl4 = sl.reshape(NT, NCH, W)
        tr[:, :, 0, :] = cl4[0:128].transpose(1, 0, 2)
        tr[:, :, 1, :] = sl4[0:128].transpose(1, 0, 2)
        tr[:, 0:64, 2, :] = cl4[128:NT].transpose(1, 0, 2)
        tr[:, 0:64, 3, :] = sl4[128:NT].transpose(1, 0, 2)
        m = {
            "xa": np.ascontiguousarray(xa),
            "tr": tr.reshape(NCH, 128, 4 * W),
            "kn": kn,
            "wq": wq_p,
            "wv": wv_p,
            "wo": wo_p,
        }
        if has_bias:
            m["xb"] = xbr
            m["wqb"] = bqA.astype(bf).reshape(1, E)
            m["wvb"] = bv.astype(bf).reshape(1, E)
        in_maps.append(m)

    want_trace = bool(int(os.environ.get("PLATCONV_TRACE", "0")))
    if want_trace:
        want_trace = _ensure_ntff_hook()
    res = run_bass_kernel_spmd(
        nc,
        in_maps,
        core_ids=list(range(NCORES)),
        trace=want_trace,
    )
    last_exec_time_ns = res.exec_time_ns
    global last_results
    last_results = res

    out = np.zeros((N, C), dtype=np.float32)
    for d in range(NCORES):
        ot = np.asarray(res.results[d]["outt"]).astype(np.float32)
        # ot: [128, 3*NUSED]; graph lj at cols 3*offs[lj], layout [3, slot]
        for lj in range(GPD):
            gb = gmap[d][lj]
            s, e_, cnt = starts[gb], starts[gb + 1], counts[gb]
            if cnt == 0 or slots[lj] == 0:
                continue
            blk = ot[:, 3 * offs[lj] : 3 * offs[lj] + 3 * slots[lj]].reshape(
                128, 3, slots[lj]
            )
            out[s:e_] = blk[:, :, :cnt].transpose(2, 1, 0).reshape(cnt, C)
    out += bo[None, :]
    return out
